# revision 16
# baseline (speedup 1.0000x reference)
"""Trainium2 Bass kernel for nn_DGCN (gnn_message_passing).

Sharding: 8 shards = (batch b in 0..3, row-half h in 0..1). Each core gets
the full 2048-node K-side tensors of its batch with the node axis ROTATED
by h*1024 so the adjacency diagonal lands at the same tile position on
every core (uniform SPMD program); the core computes rows 0..1023 of the
rotated order, which are rows [h*1024, (h+1)*1024) of the original order.

v4 — breadth-first emission (engine queues are strict FIFO; depth-first
emission head-of-line blocks every queue), row-layout LN statistics:
 - All heavy tensors bf16; host pre-transposes x/last/orig; small params
   arrive in two packed mega-tiles (one bf16, one fp32) = 2 DMAs.
 - Head-mix scalars c_h and the Hg-LayerNorm gain fold into q_w/k_w; the
   LN shift becomes an extra contraction row of an augmented [66 x 256]
   weight (rhs rows = [Hg*a ; c ; 1]).
 - Per-node LN stats (Hg, xo, att/soc rows) are computed as [1/2, 512]
   PSUM rows by matmuls against ones/selector weights, scattered into
   [8/16, 128] chunk layout by SBUF-SBUF DMA for the pointwise math, and
   scattered back as broadcast rows fed to K=1 matmuls.
 - xo LayerNorm runs in T layout (feature axis on partitions) with the
   gain applied as a per-partition activation scale; no row-major xo.
 - diag(L)_i = relu(bracket)_ii / rowsum_j relu(bracket)_ij is invariant
   to positive per-row scales, so the 1/sdA row scale of the fused
   pre-relu matrix cancels; only the x3 lhsT rows carry rsS/rsA and the
   stat rows carry {-muA, -rho*muS, sdA}.
 - relu row-sums via accum_out; GCN biases ride the next layer's matmul
   as host-folded b@W rows against a constant ones row.
 - Phase I is software-pipelined two chunks deep over all 8 PSUM banks.
"""

import sys

if '/opt/trn_rl_repo' not in sys.path:
    sys.path.insert(0, '/opt/trn_rl_repo')

from contextlib import ExitStack

import numpy as np
import ml_dtypes

import concourse.bass as bass
import concourse.tile as tile
from concourse import bacc, mybir
from concourse.bass_interp import get_hw_module
from concourse.bass_utils import run_bass_kernel_spmd

FP = mybir.dt.float32
BF = mybir.dt.bfloat16
AL = mybir.AluOpType
AF = mybir.ActivationFunctionType
AX = mybir.AxisListType

B, N, E, G, H = 4, 2048, 64, 64, 4
D = H * G          # 256
HALF = N // 2      # own rows per core
NCH = N // 128     # 16 chunks over all nodes
HCH = HALF // 128  # 8 own chunks
MB = 512
NMB = N // MB      # 4
EPS = 1e-5

# wpack (bf16 [128, WPACK_W]) column layout
W_IDB, W_WZ, W_WR, W_WH = 0, 128, 192, 256
W_QA, W_KA = 320, 576
W_FC1, W_FC2, W_FC3A = 832, 848, 850
W_W1A, W_W2A, W_W3A = 914, 978, 1042
W_SEL, W_ONE = 1106, 1108
WPACK_W = 1280
# fpack (fp32 [128, FPACK_W]) column layout
F_IDF, F_B, F_EPS, F_XG, F_XB3, F_BN = 0, 128, 132, 133, 134, 136
FPACK_W = 528

_CACHE = {}


def _tp(nc, out_ap, in_ap, ident):
    k = in_ap.partition_size()
    nc.tensor.transpose(out_ap, in_ap, ident[0:k, 0:k])


def _leaky(nc, dst):
    nc.vector.scalar_tensor_tensor(dst, dst, 0.01, dst, AL.mult, AL.max)


def _ln_rows(nc, pool, t_in, t_out, g_b, b_b, ngr, tag, epsc):
    """LayerNorm over 64-wide groups: t_in [128, ngr*64] -> t_out."""
    a3 = t_in[:].rearrange("p (g e) -> p g e", e=64)
    o3 = t_out[:].rearrange("p (g e) -> p g e", e=64)
    sm = pool.tile([128, ngr], FP, tag=f"{tag}_sm")
    nc.vector.tensor_reduce(sm[:], a3, AX.X, AL.add)
    sq = pool.tile([128, ngr * 64], FP, tag=f"{tag}_sq")
    nc.scalar.square(sq[:], t_in[:])
    sqs = pool.tile([128, ngr], FP, tag=f"{tag}_sqs")
    nc.vector.tensor_reduce(sqs[:], sq[:].rearrange("p (g e) -> p g e", e=64),
                            AX.X, AL.add)
    mu = pool.tile([128, ngr], FP, tag=f"{tag}_mu")
    nc.vector.tensor_scalar(mu[:], sm[:], 1.0 / 64, None, AL.mult)
    mu2 = pool.tile([128, ngr], FP, tag=f"{tag}_mu2")
    nc.vector.tensor_tensor(mu2[:], mu[:], mu[:], AL.mult)
    var = pool.tile([128, ngr], FP, tag=f"{tag}_var")
    nc.vector.scalar_tensor_tensor(var[:], sqs[:], 1.0 / 64, mu2[:], AL.mult,
                                   AL.subtract)
    sd = pool.tile([128, ngr], FP, tag=f"{tag}_sd")
    nc.scalar.activation(sd[:], var[:], AF.Sqrt, bias=epsc)
    rs = pool.tile([128, ngr], FP, tag=f"{tag}_rs")
    nc.vector.reciprocal(rs[:], sd[:])
    mu_b = mu[:].unsqueeze(2).broadcast_to([128, ngr, 64])
    rs_b = rs[:].unsqueeze(2).broadcast_to([128, ngr, 64])
    g3 = g_b.unsqueeze(1).broadcast_to([128, ngr, 64])
    b3 = b_b.unsqueeze(1).broadcast_to([128, ngr, 64])
    xc = pool.tile([128, ngr * 64], FP, tag=f"{tag}_xc")
    xc3 = xc[:].rearrange("p (g e) -> p g e", e=64)
    nc.vector.tensor_tensor(xc3, a3, mu_b, AL.subtract)
    nc.vector.tensor_tensor(xc3, xc3, rs_b, AL.mult)
    nc.vector.tensor_tensor(xc3, xc3, g3, AL.mult)
    nc.vector.tensor_tensor(o3, xc3, b3, AL.add)


def _stat_land(nc, small, sum_t, sq_t, nch, tag, epsc, inv):
    """[nch,128] sums/sumsq -> (a, c) = (1/sd, -mu/sd), both [nch, 128] fp32."""
    mu = small.tile([nch, 128], FP, tag=f"{tag}_mu", name=f"{tag}_mu")
    nc.vector.tensor_scalar(mu[:], sum_t, inv, None, AL.mult)
    mu2 = small.tile([nch, 128], FP, tag=f"{tag}_mu2", name=f"{tag}_mu2")
    nc.vector.tensor_tensor(mu2[:], mu[:], mu[:], AL.mult)
    var = small.tile([nch, 128], FP, tag=f"{tag}_var", name=f"{tag}_var")
    nc.vector.scalar_tensor_tensor(var[:], sq_t, inv, mu2[:], AL.mult, AL.subtract)
    sd = small.tile([nch, 128], FP, tag=f"{tag}_sd", name=f"{tag}_sd")
    nc.scalar.activation(sd[:], var[:], AF.Sqrt, bias=epsc)
    a = small.tile([nch, 128], FP, tag=f"{tag}_a", name=f"{tag}_a")
    nc.vector.reciprocal(a[:], sd[:])
    c = small.tile([nch, 128], FP, tag=f"{tag}_c", name=f"{tag}_c")
    nc.vector.scalar_tensor_tensor(c[:], mu[:], -1.0, a[:], AL.mult, AL.mult)
    return mu, sd, a, c


def _emit(ctx: ExitStack, tc: tile.TileContext, io: dict):
    nc = tc.nc

    persist = ctx.enter_context(tc.tile_pool(name="persist", bufs=1))
    small = ctx.enter_context(tc.tile_pool(name="small", bufs=1))

    # ---------------- packed params (2 DMAs) ----------------
    wp = persist.tile([128, WPACK_W], BF, tag="wp")
    nc.sync.dma_start(wp[:], io["wpack"][:])
    fp_ = persist.tile([128, FPACK_W], FP, tag="fp_")
    nc.scalar.dma_start(fp_[:], io["fpack"][:])

    identb = wp[:, W_IDB:W_IDB + 128]
    wz = wp[:, W_WZ:W_WZ + 64]
    wr = wp[:, W_WR:W_WR + 64]
    wh = wp[:, W_WH:W_WH + 64]
    kA = wp[0:66, W_KA:W_KA + 256]
    fc1s = wp[0:64, W_FC1:W_FC1 + 16]
    fc2s = wp[0:16, W_FC2:W_FC2 + 2]
    fc3s = wp[0:2, W_FC3A:W_FC3A + 64]
    fc3a = wp[0:3, W_FC3A:W_FC3A + 64]
    w1a = wp[0:65, W_W1A:W_W1A + 64]
    w2a = wp[0:65, W_W2A:W_W2A + 64]
    w3a = wp[0:65, W_W3A:W_W3A + 64]
    sel2 = wp[:, W_SEL:W_SEL + 2]
    ones128c = wp[:, W_ONE:W_ONE + 1]
    ones64c = wp[0:64, W_ONE:W_ONE + 1]
    onesr128 = wp[0:1, W_ONE:W_ONE + 128]
    onesr64 = wp[0:1, W_ONE:W_ONE + 64]

    ident = fp_[:, F_IDF:F_IDF + 128]
    fc1b = fp_[0:16, F_B + 0:F_B + 1]
    fc2b = fp_[0:2, F_B + 1:F_B + 2]
    fc3b = fp_[0:64, F_B + 2:F_B + 3]
    epsc128 = fp_[0:128, F_EPS:F_EPS + 1]
    epsc16 = fp_[0:16, F_EPS:F_EPS + 1]
    epsc8 = fp_[0:8, F_EPS:F_EPS + 1]
    xng_c = fp_[0:64, F_XG:F_XG + 1]
    xb3_c = fp_[0:64, F_XB3:F_XB3 + 1]

    # LN parameter rows -> [128, 64] broadcast tiles via gpsimd (small)
    brows = {}
    for k, nm in enumerate(("bng", "bnb", "lng", "lnb")):
        t = persist.tile([128, 64], FP, tag=f"{nm}_b", name=f"{nm}_b")
        nc.gpsimd.partition_broadcast(
            t[:], fp_[0:1, F_BN + 64 * k:F_BN + 64 * (k + 1)])
        brows[nm] = t

    # ---------------- big persistent tiles ----------------
    xT = persist.tile([64, N], BF, tag="xT")
    lastT = persist.tile([64, N], BF, tag="lastT")
    c1 = persist.tile([128, N], BF, tag="c1")      # [x3 ; last]
    c2 = persist.tile([128, N], BF, tag="c2")      # [r*last ; x3]
    hgsq = persist.tile([128, N], BF, tag="hgsq")  # [Hg_raw ; Hg_raw^2]
    HgQ = persist.tile([66, N], BF, tag="HgQ")     # [Hg*a ; c ; 1]
    osq = persist.tile([128, HALF], BF, tag="osq")  # [origT ; origT^2]
    a_row = persist.tile([1, N], BF, tag="a_row")
    kT0 = persist.tile([128, N], BF, tag="kT0")
    kT1 = persist.tile([128, N], BF, tag="kT1")
    k2T0 = persist.tile([128, N], BF, tag="k2T0")
    k2T1 = persist.tile([128, N], BF, tag="k2T1")
    qT0 = persist.tile([128, HALF], BF, tag="qT0")
    qT1 = persist.tile([128, HALF], BF, tag="qT1")
    x3gs = persist.tile([67, N], BF, tag="x3gs")   # [x3*gs ; ga ; gs ; cb]
    x3rA = persist.tile([67, HALF], BF, tag="x3rA")
    ga_b = persist.tile([128, N], BF, tag="ga_b")
    gs_b = persist.tile([64, N], BF, tag="gs_b")
    ga_r = persist.tile([1, N], BF, tag="ga_r")
    gs_r = persist.tile([1, N], BF, tag="gs_r")
    gt_sb = persist.tile([128, 256], BF, tag="gt_sb")
    gb_sb = persist.tile([128, 256], BF, tag="gb_sb")
    gs_f = persist.tile([64, 64], BF, tag="gs_f")
    ks0 = persist.tile([128, 1], BF, tag="ks0")
    ks1 = persist.tile([128, 1], BF, tag="ks1")
    xsb = persist.tile([64, 1], BF, tag="xsb")
    rc32 = persist.tile([128, 4 * HCH], FP, tag="rc32")
    dg8 = persist.tile([128, HCH], FP, tag="dg8")
    dl = persist.tile([128, HCH], FP, tag="dl")
    x1T = persist.tile([16, N], BF, tag="x1T")
    x2a = persist.tile([3, N], BF, tag="x2a")      # [x2 ; 1]
    e0sb = persist.tile([128, HALF], BF, tag="e0sb")
    e1sb = persist.tile([128, HALF], BF, tag="e1sb")
    essb = persist.tile([64, HALF], BF, tag="essb")
    ph1sb = persist.tile([64, HALF], BF, tag="ph1sb")
    finsq = persist.tile([128, HALF], BF, tag="finsq")
    cT_sb = persist.tile([128, NCH], FP, tag="cT_sb")
    lastR = persist.tile([128, HCH * 64], FP, tag="lastR")
    x1aug = persist.tile([65, HALF], BF, tag="x1aug")  # [xo^T + b3 ; 1]
    hca = persist.tile([65, HALF], BF, tag="hca")
    hcb = persist.tile([65, HALF], BF, tag="hcb")
    fin = persist.tile([128, HCH * 64], FP, tag="fin")

    # input loads
    nc.sync.dma_start(xT[:], io["xT"][:])
    nc.sync.dma_start(lastT[:], io["lastT"][:])
    nc.sync.dma_start(c1[64:128, :], io["lastT"][:])
    nc.sync.dma_start(osq[0:64, :], io["origT"][:])
    nc.sync.dma_start(x3gs[64:67, :], io["corr4"][0:3, :])
    nc.sync.dma_start(ga_r[:], io["corr4"][0:1, :])
    nc.sync.dma_start(gs_r[:], io["corr4"][1:2, :])
    # constant-ones rows
    nc.gpsimd.dma_start(HgQ[65:66, :], io["corr4"][3:4, :])
    nc.gpsimd.dma_start(x2a[2:3, :], io["corr4"][3:4, :])
    nc.gpsimd.dma_start(x1aug[64:65, :], io["corr4"][3:4, 0:HALF])
    nc.gpsimd.dma_start(hca[64:65, :], io["corr4"][3:4, 0:HALF])
    nc.gpsimd.dma_start(hcb[64:65, :], io["corr4"][3:4, 0:HALF])

    frontA = ExitStack()
    fps = frontA.enter_context(tc.tile_pool(name="fps", bufs=6, space="PSUM"))
    gw = frontA.enter_context(tc.tile_pool(name="gw", bufs=4))

    MBs = [slice(j * MB, (j + 1) * MB) for j in range(NMB)]
    HBs = [slice(j * MB, (j + 1) * MB) for j in range(2)]

    # ---- ga / gs broadcast tiles via K=1 matmuls (breadth) ----
    gps_ = [fps.tile([128, MB], FP, tag="fp", name=f"gab_{j}") for j in range(NMB)]
    gss_ = [fps.tile([64, MB], FP, tag="fp", name=f"gsb_{j}") for j in range(2)]
    for j in range(NMB):
        nc.tensor.matmul(gps_[j][:], onesr128, ga_r[:, MBs[j]], start=True, stop=True)
    for j in range(2):
        nc.tensor.matmul(gss_[j][:], onesr64, gs_r[:, j * MB:(j + 1) * MB],
                         start=True, stop=True)
    for j in range(NMB):
        (nc.vector.tensor_copy if j % 2 == 0 else nc.scalar.copy)(
            ga_b[:, MBs[j]], gps_[j][:])
    for j in range(2):
        (nc.scalar.copy if j % 2 == 0 else nc.vector.tensor_copy)(
            gs_b[:, slice(j * MB, (j + 1) * MB)], gss_[j][:])
    gss2_ = [fps.tile([64, MB], FP, tag="fp", name=f"gsb2_{j}") for j in range(2)]
    for j in range(2):
        sl = slice((2 + j) * MB, (3 + j) * MB)
        nc.tensor.matmul(gss2_[j][:], onesr64, gs_r[:, sl], start=True, stop=True)
        (nc.vector.tensor_copy if j % 2 == 0 else nc.scalar.copy)(
            gs_b[:, sl], gss2_[j][:])

    # ---- xo stats (input-only dependent, fills the early pipeline) ----
    nc.scalar.square(osq[64:128, :], osq[0:64, :])
    oxp = [fps.tile([2, MB], FP, tag="fp", name=f"oxp_{j}") for j in range(2)]
    for j in range(2):
        nc.tensor.matmul(oxp[j][:], sel2, osq[:, HBs[j]], start=True, stop=True)
    oxs = small.tile([2, HALF], FP, tag="oxs")
    for j in range(2):
        (nc.vector.tensor_copy if j == 0 else nc.scalar.copy)(oxs[:, HBs[j]], oxp[j][:])
    oxs0 = small.tile([HCH, 128], FP, tag="oxs0")
    nc.sync.dma_start(oxs0[:], oxs[0:1, :].rearrange("o (i p) -> o i p", p=128))
    oxs1 = small.tile([HCH, 128], FP, tag="oxs1")
    nc.sync.dma_start(oxs1[:], oxs[1:2, :].rearrange("o (i p) -> o i p", p=128))
    _, _, oa, oc = _stat_land(nc, small, oxs0[:], oxs1[:], HCH, "ox", epsc8, 1.0 / 64)
    oa8 = small.tile([HCH, 128], BF, tag="oa8")
    nc.vector.tensor_copy(oa8[:], oa[:])
    oc8 = small.tile([HCH, 128], BF, tag="oc8")
    nc.scalar.copy(oc8[:], oc[:])
    oar = small.tile([1, HALF], BF, tag="oar")
    nc.sync.dma_start(oar[:].rearrange("o (i p) -> o i p", p=128), oa8[:])
    ocr = small.tile([1, HALF], BF, tag="ocr")
    nc.sync.dma_start(ocr[:].rearrange("o (i p) -> o i p", p=128), oc8[:])

    # ============ hyper fc stack (breadth-first stages) ============
    xacc = small.tile([64, NMB], FP, tag="xacc")
    p1 = [fps.tile([16, MB], FP, tag="fp", name=f"p1_{j}") for j in range(NMB)]
    for j in range(NMB):
        nc.tensor.matmul(p1[j][:], fc1s, xT[:, MBs[j]], start=True, stop=True)
    for j in range(NMB):
        nc.scalar.activation(x1T[:, MBs[j]], p1[j][:], AF.Sigmoid, bias=fc1b)
    p2 = [fps.tile([2, MB], FP, tag="fp", name=f"p2_{j}") for j in range(NMB)]
    for j in range(NMB):
        nc.tensor.matmul(p2[j][:], fc2s, x1T[:, MBs[j]], start=True, stop=True)
    for j in range(NMB):
        nc.scalar.activation(x2a[0:2, MBs[j]], p2[j][:], AF.Sigmoid, bias=fc2b)
    p3 = [fps.tile([64, MB], FP, tag="fp", name=f"p3_{j}") for j in range(NMB)]
    for j in range(NMB):
        nc.tensor.matmul(p3[j][:], fc3s, x2a[0:2, MBs[j]], start=True, stop=True)
    for j in range(NMB):
        nc.scalar.activation(c1[0:64, MBs[j]], p3[j][:], AF.Identity, bias=fc3b,
                             accum_out=xacc[:, j:j + 1])
    for j in range(NMB):
        nc.gpsimd.tensor_copy(c2[64:128, MBs[j]], c1[0:64, MBs[j]])
    xs_f = small.tile([64, 1], FP, tag="xs_f")
    nc.vector.tensor_reduce(xs_f[:], xacc[:], AX.X, AL.add)
    nc.vector.tensor_copy(xsb[:], xs_f[:])

    # ================= GRU gates (breadth-first stages) =================
    zp = [fps.tile([64, MB], FP, tag="fp", name=f"zp_{j}") for j in range(NMB)]
    for j in range(NMB):
        nc.tensor.matmul(zp[j][:], wz, c1[:, MBs[j]], start=True, stop=True)
    zt = [gw.tile([64, MB], BF, tag="zt", name=f"zt_{j}") for j in range(NMB)]
    for j in range(NMB):
        nc.scalar.activation(zt[j][:], zp[j][:], AF.Sigmoid)
    rp = [fps.tile([64, MB], FP, tag="fp", name=f"rp_{j}") for j in range(NMB)]
    for j in range(NMB):
        nc.tensor.matmul(rp[j][:], wr, c1[:, MBs[j]], start=True, stop=True)
    rt = [gw.tile([64, MB], BF, tag="rt", name=f"rt_{j}") for j in range(NMB)]
    for j in range(NMB):
        nc.scalar.activation(rt[j][:], rp[j][:], AF.Sigmoid)
    for j in range(NMB):
        nc.gpsimd.tensor_tensor(c2[0:64, MBs[j]], rt[j][:], lastT[:, MBs[j]], AL.mult)
    hp = [fps.tile([64, MB], FP, tag="fp", name=f"hp_{j}") for j in range(NMB)]
    for j in range(NMB):
        nc.tensor.matmul(hp[j][:], wh, c2[:, MBs[j]], start=True, stop=True)
    ht = [gw.tile([64, MB], BF, tag="ht", name=f"ht_{j}") for j in range(NMB)]
    for j in range(NMB):
        nc.scalar.activation(ht[j][:], hp[j][:], AF.Tanh)
    dt_ = [gw.tile([64, MB], BF, tag="dt", name=f"dt_{j}") for j in range(NMB)]
    for j in range(NMB):
        nc.vector.tensor_tensor(dt_[j][:], ht[j][:], lastT[:, MBs[j]], AL.subtract)
    for j in range(NMB):
        nc.vector.tensor_tensor(dt_[j][:], dt_[j][:], zt[j][:], AL.mult)
    for j in range(NMB):
        nc.vector.tensor_tensor(hgsq[0:64, MBs[j]], dt_[j][:], lastT[:, MBs[j]], AL.add)

    # Hg^2 (scalar) then Hg LN stats rows
    nc.scalar.square(hgsq[64:128, 0:HALF], hgsq[0:64, 0:HALF])
    nc.vector.tensor_tensor(hgsq[64:128, HALF:N], hgsq[0:64, HALF:N],
                            hgsq[0:64, HALF:N], AL.mult)
    hsp = [fps.tile([2, MB], FP, tag="fp", name=f"hsp_{j}") for j in range(NMB)]
    for j in range(NMB):
        nc.tensor.matmul(hsp[j][:], sel2, hgsq[:, MBs[j]], start=True, stop=True)
    hsum = small.tile([2, N], FP, tag="hsum")
    for j in range(NMB):
        (nc.vector.tensor_copy if j % 2 == 0 else nc.scalar.copy)(
            hsum[:, MBs[j]], hsp[j][:])
    hst0 = small.tile([NCH, 128], FP, tag="hst0")
    nc.sync.dma_start(hst0[:], hsum[0:1, :].rearrange("o (i p) -> o i p", p=128))
    hst1 = small.tile([NCH, 128], FP, tag="hst1")
    nc.sync.dma_start(hst1[:], hsum[1:2, :].rearrange("o (i p) -> o i p", p=128))
    _, _, ha, hc = _stat_land(nc, small, hst0[:], hst1[:], NCH, "hg", epsc16, 1.0 / 64)
    ha16 = small.tile([NCH, 128], BF, tag="ha16")
    nc.vector.tensor_copy(ha16[:], ha[:])
    hc16 = small.tile([NCH, 128], BF, tag="hc16")
    nc.scalar.copy(hc16[:], hc[:])
    nc.sync.dma_start(a_row[:].rearrange("o (i p) -> o i p", p=128), ha16[:])
    nc.sync.dma_start(HgQ[64:65, :].rearrange("o (i p) -> o i p", p=128), hc16[:])
    # c in chunk-column layout for the lastH bias path
    pcc = fps.tile([128, NCH], FP, tag="fp", name="pcc", padded_shape=[128, 512])
    _tp(nc, pcc[:], hc[:], ident)
    nc.scalar.copy(cT_sb[:], pcc[:])

    # HgA = Hg_raw * a (K=1 broadcast matmul + fused multiply from PSUM)
    ab = [fps.tile([64, MB], FP, tag="fp", name=f"ab_{j}") for j in range(NMB)]
    for j in range(NMB):
        nc.tensor.matmul(ab[j][:], onesr64, a_row[:, MBs[j]], start=True, stop=True)
    for j in range(NMB):
        nc.vector.tensor_tensor(HgQ[0:64, MBs[j]], hgsq[0:64, MBs[j]], ab[j][:],
                                AL.mult)

    # ===================== q / k projections =====================
    kacc = small.tile([128, 8], FP, tag="kacc")
    kjobs = []
    for half, dst in ((0, kT0), (1, kT1)):
        for j in range(NMB):
            kjobs.append((dst, slice(W_KA + 128 * half, W_KA + 128 * (half + 1)),
                          MBs[j], kacc[:, 4 * half + j:4 * half + j + 1]))
    qjobs = []
    for half, dst in ((0, qT0), (1, qT1)):
        for j in range(2):
            qjobs.append((dst, slice(W_QA + 128 * half, W_QA + 128 * (half + 1)),
                          HBs[j], None))
    kq_ps = []
    for idx, (dst, wsl, sl, acc) in enumerate(kjobs + qjobs):
        kp = fps.tile([128, MB], FP, tag="fp", name=f"kqp_{idx}")
        nc.tensor.matmul(kp[:], wp[0:66, wsl], HgQ[:, sl], start=True, stop=True)
        kq_ps.append(kp)
    for idx, (dst, wsl, sl, acc) in enumerate(kjobs + qjobs):
        nc.scalar.copy(dst[:, sl], kq_ps[idx][:])
        if acc is not None:
            nc.vector.scalar_tensor_tensor(dst[:, sl], dst[:, sl], 0.01, dst[:, sl],
                                           AL.mult, AL.max, accum_out=acc)
        else:
            _leaky(nc, dst[:, sl])
    ks_f = small.tile([128, 2], FP, tag="ks_f")
    nc.vector.tensor_reduce(ks_f[:], kacc[:].rearrange("p (h j) -> p h j", j=4),
                            AX.X, AL.add)
    nc.vector.tensor_copy(ks0[:], ks_f[:, 0:1])
    nc.vector.tensor_copy(ks1[:], ks_f[:, 1:2])
    # k2 = k * ga ; x3gs rows 0:64 = x3 * gs
    nc.vector.tensor_tensor(k2T0[:], kT0[:], ga_b[:], AL.mult)
    nc.vector.tensor_tensor(k2T1[:], kT1[:], ga_b[:], AL.mult)
    nc.vector.tensor_tensor(x3gs[0:64, :], c1[0:64, :], gs_b[:], AL.mult)

    frontA.close()

    # ===================== Gram matrices =====================
    with tc.tile_pool(name="gpsp", bufs=3, space="PSUM") as gpsp, \
         tc.tile_pool(name="krpp", bufs=3, space="PSUM") as krpp, \
         tc.tile_pool(name="krp", bufs=3) as krp:
        gt_ps = gpsp.tile([128, 256], FP, tag="g", padded_shape=[128, 512])
        gb_ps = gpsp.tile([128, 256], FP, tag="g", padded_shape=[128, 512])
        for mi in range(NCH):
            msl = slice(mi * 128, (mi + 1) * 128)
            krq = krpp.tile([128, 256], FP, tag="kr", padded_shape=[128, 512])
            nc.tensor.matmul(krq[:], HgQ[:, msl], kA, start=True, stop=True)
            kr = krp.tile([128, 256], BF, tag="kr")
            nc.scalar.copy(kr[:], krq[:])
            _leaky(nc, kr[:])
            nc.tensor.matmul(gt_ps[:], kr[:, 0:128], kr[:],
                             start=(mi == 0), stop=(mi == NCH - 1))
            nc.tensor.matmul(gb_ps[:], kr[:, 128:256], kr[:],
                             start=(mi == 0), stop=(mi == NCH - 1))
        nc.vector.tensor_copy(gt_sb[:], gt_ps[:])
        nc.scalar.copy(gb_sb[:], gb_ps[:])
        gs_ps = gpsp.tile([64, 64], FP, tag="g", padded_shape=[64, 512])
        for mi in range(NCH):
            msl = slice(mi * 128, (mi + 1) * 128)
            xrq = krpp.tile([128, 64], FP, tag="kr", padded_shape=[128, 512])
            nc.tensor.matmul(xrq[:], x2a[:, msl], fc3a, start=True, stop=True)
            xr = krp.tile([128, 64], BF, tag="xr")
            (nc.vector.tensor_copy if mi % 2 == 0 else nc.scalar.copy)(xr[:], xrq[:])
            nc.tensor.matmul(gs_ps[:], xr[:], xr[:],
                             start=(mi == 0), stop=(mi == NCH - 1))
        nc.vector.tensor_copy(gs_f[:], gs_ps[:])

    # ============== own-row stats: S1, T1, S2, T2 rows ==============
    statq = ExitStack()
    ups = statq.enter_context(tc.tile_pool(name="ups", bufs=2, space="PSUM"))
    sps = statq.enter_context(tc.tile_pool(name="sps", bufs=2, space="PSUM"))
    lps = statq.enter_context(tc.tile_pool(name="lps", bufs=2, space="PSUM"))
    # u = G q per 512-half; e = u * q  (separate e0/e1, summed in PSUM below)
    for jb in range(2):
        sl = HBs[jb]
        ut0 = ups.tile([128, MB], FP, tag="ut", name=f"ut0_{jb}")
        nc.tensor.matmul(ut0[:], gt_sb[:, 0:128], qT0[:, sl], start=True, stop=False)
        nc.tensor.matmul(ut0[:], gb_sb[:, 0:128], qT1[:, sl], start=False, stop=True)
        ut1 = ups.tile([128, MB], FP, tag="ut", name=f"ut1_{jb}")
        nc.tensor.matmul(ut1[:], gt_sb[:, 128:256], qT0[:, sl], start=True, stop=False)
        nc.tensor.matmul(ut1[:], gb_sb[:, 128:256], qT1[:, sl], start=False, stop=True)
        nc.vector.tensor_tensor(e0sb[:, sl], ut0[:], qT0[:, sl], AL.mult)
        nc.vector.tensor_tensor(e1sb[:, sl], ut1[:], qT1[:, sl], AL.mult)
    for jb in range(2):
        sl = HBs[jb]
        us = ups.tile([64, MB], FP, tag="ut", name=f"us_{jb}")
        nc.tensor.matmul(us[:], gs_f[:], c1[0:64, sl], start=True, stop=True)
        nc.vector.tensor_tensor(essb[:, sl], us[:], c1[0:64, sl], AL.mult)

    s1sb = small.tile([1, HALF], FP, tag="s1sb")
    t1sb = small.tile([1, HALF], FP, tag="t1sb")
    s2sb = small.tile([1, HALF], FP, tag="s2sb")
    t2sb = small.tile([1, HALF], FP, tag="t2sb")
    for jb in range(2):
        sl = HBs[jb]
        s1p = sps.tile([1, MB], FP, tag="st", name=f"s1p_{jb}", padded_shape=[1, 512])
        nc.tensor.matmul(s1p[:], ks0[:], qT0[:, sl], start=True, stop=False)
        nc.tensor.matmul(s1p[:], ks1[:], qT1[:, sl], start=False, stop=True)
        t1p = sps.tile([1, MB], FP, tag="st", name=f"t1p_{jb}", padded_shape=[1, 512])
        nc.tensor.matmul(t1p[:], xsb[:], c1[0:64, sl], start=True, stop=True)
        nc.scalar.copy(s1sb[:, sl], s1p[:])
        nc.vector.tensor_copy(t1sb[:, sl], t1p[:])
    for jb in range(2):
        sl = HBs[jb]
        s2p = sps.tile([1, MB], FP, tag="st", name=f"s2p_{jb}", padded_shape=[1, 512])
        nc.tensor.matmul(s2p[:], ones128c, e0sb[:, sl], start=True, stop=False)
        nc.tensor.matmul(s2p[:], ones128c, e1sb[:, sl], start=False, stop=True)
        t2p = sps.tile([1, MB], FP, tag="st", name=f"t2p_{jb}", padded_shape=[1, 512])
        nc.tensor.matmul(t2p[:], ones64c, essb[:, sl], start=True, stop=True)
        nc.scalar.copy(s2sb[:, sl], s2p[:])
        nc.vector.tensor_copy(t2sb[:, sl], t2p[:])
    s1t = small.tile([HCH, 128], FP, tag="s1t")
    nc.sync.dma_start(s1t[:], s1sb[:].rearrange("o (i p) -> o i p", p=128))
    t1t = small.tile([HCH, 128], FP, tag="t1t")
    nc.scalar.dma_start(t1t[:], t1sb[:].rearrange("o (i p) -> o i p", p=128))
    s2t = small.tile([HCH, 128], FP, tag="s2t")
    nc.sync.dma_start(s2t[:], s2sb[:].rearrange("o (i p) -> o i p", p=128))
    t2t = small.tile([HCH, 128], FP, tag="t2t")
    nc.scalar.dma_start(t2t[:], t2sb[:].rearrange("o (i p) -> o i p", p=128))

    # ---- fillers for the stats-land latency ----
    # lastH output (Hg LN rows, own half)
    for i in range(HCH):
        pt = lps.tile([128, 64], BF, tag="lpt", name=f"lpt_{i}",
                      padded_shape=[128, 1024])
        _tp(nc, pt[:], HgQ[0:64, i * 128:(i + 1) * 128], identb)
        nc.scalar.activation(lastR[:, i * 64:(i + 1) * 64], pt[:], AF.Identity,
                             bias=cT_sb[:, i:i + 1])
    l3 = lastR[:].rearrange("p (g e) -> p g e", e=64)
    lg3 = brows["bng"][:].unsqueeze(1).broadcast_to([128, HCH, 64])
    lb3 = brows["bnb"][:].unsqueeze(1).broadcast_to([128, HCH, 64])
    nc.vector.tensor_tensor(l3, l3, lg3, AL.mult)
    nc.vector.tensor_tensor(l3, l3, lb3, AL.add)
    nc.sync.dma_start(io["lastH"].rearrange("(i p) e -> p i e", p=128),
                      lastR[:].rearrange("p (i e) -> p i e", e=64))

    # xo affine into x1aug
    oab = [sps.tile([64, MB], FP, tag="st", name=f"oab_{j}") for j in range(2)]
    for j in range(2):
        nc.tensor.matmul(oab[j][:], onesr64, oar[:, HBs[j]], start=True, stop=True)
    ocb = [sps.tile([64, MB], FP, tag="st", name=f"ocb_{j}") for j in range(2)]
    for j in range(2):
        nc.tensor.matmul(ocb[j][:], onesr64, ocr[:, HBs[j]], start=True, stop=True)
    for j in range(2):
        tb = small.tile([64, MB], BF, tag=f"oxt_{j}", name=f"oxt_{j}")
        nc.vector.tensor_tensor(tb[:], osq[0:64, HBs[j]], oab[j][:], AL.mult)
        nc.vector.tensor_tensor(tb[:], tb[:], ocb[j][:], AL.add)
        nc.scalar.activation(x1aug[0:64, HBs[j]], tb[:], AF.Identity,
                             scale=xng_c, bias=xb3_c)

    # GCN layer-1 matmul (dl-independent)
    for jb in range(2):
        ph1 = sps.tile([64, MB], FP, tag="st", name=f"ph1_{jb}")
        nc.tensor.matmul(ph1[:], w1a, x1aug[:, HBs[jb]], start=True, stop=True)
        (nc.scalar.copy if jb == 0 else nc.vector.tensor_copy)(
            ph1sb[:, HBs[jb]], ph1[:])

    # ---- own stats land [8, 128] ----
    muA, sdA, rsA_, _cA = _stat_land(nc, small, s1t[:], s2t[:], HCH, "sa",
                                     epsc8, 1.0 / N)
    muS, sdS, rsS_, _cS = _stat_land(nc, small, t1t[:], t2t[:], HCH, "ss",
                                     epsc8, 1.0 / N)
    rho = small.tile([HCH, 128], FP, tag="rho")
    nc.vector.tensor_tensor(rho[:], rsS_[:], sdA[:], AL.mult)
    r64t = small.tile([HCH, 128], BF, tag="r64t")
    nc.vector.tensor_scalar(r64t[:], muA[:], -1.0, None, AL.mult)
    r65t = small.tile([HCH, 128], BF, tag="r65t")
    nc.vector.scalar_tensor_tensor(r65t[:], muS[:], -1.0, rho[:], AL.mult, AL.mult)
    r66t = small.tile([HCH, 128], BF, tag="r66t")
    nc.scalar.copy(r66t[:], sdA[:])
    rho16 = small.tile([HCH, 128], BF, tag="rho16")
    nc.scalar.copy(rho16[:], rho[:])
    nc.sync.dma_start(x3rA[64:65, :].rearrange("o (i p) -> o i p", p=128), r64t[:])
    nc.scalar.dma_start(x3rA[65:66, :].rearrange("o (i p) -> o i p", p=128), r65t[:])
    nc.sync.dma_start(x3rA[66:67, :].rearrange("o (i p) -> o i p", p=128), r66t[:])
    rho_row = small.tile([1, HALF], BF, tag="rho_row")
    nc.scalar.dma_start(rho_row[:].rearrange("o (i p) -> o i p", p=128), rho16[:])
    for jb in range(2):
        rp_ = sps.tile([64, MB], FP, tag="st", name=f"rhob_{jb}")
        nc.tensor.matmul(rp_[:], onesr64, rho_row[:, HBs[jb]], start=True, stop=True)
        nc.vector.tensor_tensor(x3rA[0:64, HBs[jb]], c1[0:64, HBs[jb]], rp_[:],
                                AL.mult)
    statq.close()

    # =================== phase I: fused adjacency ===================
    with tc.tile_pool(name="zps", bufs=8, space="PSUM") as zps, \
         tc.tile_pool(name="scrv", bufs=2) as scrv, \
         tc.tile_pool(name="scra", bufs=2) as scra:
        ztiles = {}

        def passes12(i):
            csl = slice(i * 128, (i + 1) * 128)
            zpt = [zps.tile([128, MB], FP, tag="zpt", name=f"zp_{i}_{m}")
                   for m in range(NMB)]
            ztiles[i] = zpt
            for mb in range(NMB):
                nc.tensor.matmul(zpt[mb][:], qT0[:, csl],
                                 k2T0[:, mb * MB:(mb + 1) * MB],
                                 start=True, stop=False)
            for mb in range(NMB):
                nc.tensor.matmul(zpt[mb][:], qT1[:, csl],
                                 k2T1[:, mb * MB:(mb + 1) * MB],
                                 start=False, stop=False)

        def pass3(i):
            csl = slice(i * 128, (i + 1) * 128)
            zpt = ztiles[i]
            for mb in range(NMB):
                nc.tensor.matmul(zpt[mb][:], x3rA[:, csl],
                                 x3gs[:, mb * MB:(mb + 1) * MB],
                                 start=False, stop=True)
            for mb in range(NMB):
                acc = rc32[:, 4 * i + mb:4 * i + mb + 1]
                if mb % 2 == 0:
                    scr = scrv.tile([128, MB], FP, tag="scr", name=f"scr_{i}_{mb}")
                    nc.vector.tensor_scalar(scr[:], zpt[mb][:], 0.0, None, AL.max,
                                            AL.add, accum_out=acc)
                else:
                    scr = scra.tile([128, MB], FP, tag="scr2", name=f"scr2_{i}_{mb}")
                    nc.scalar.activation(scr[:], zpt[mb][:], AF.Relu, accum_out=acc)
                if mb == i // 4:
                    off = (i * 128) % MB
                    dsel = scrv.tile([128, 128], FP, tag="dsel", name=f"dsel_{i}")
                    nc.gpsimd.affine_select(
                        out=dsel[:], in_=scr[:, off:off + 128],
                        compare_op=AL.is_equal, fill=0.0, base=0,
                        pattern=[[-1, 128]], channel_multiplier=1)
                    nc.vector.tensor_reduce(dg8[:, i:i + 1], dsel[:], AX.X, AL.add)

        passes12(0)
        passes12(1)
        for i in range(HCH):
            pass3(i)
            if i + 2 < HCH:
                passes12(i + 2)

    # ============== per-half tail: dl -> GCN -> fin LN ==============
    epsc4 = fp_[0:4, F_EPS:F_EPS + 1]
    with tc.tile_pool(name="jps", bufs=4, space="PSUM") as jps:
        dls = small.tile([64, HALF], BF, tag="dls")
        fsum = small.tile([2, HALF], FP, tag="fsum")
        for jb in range(2):
            hsl = HBs[jb]
            ch4 = slice(4 * jb, 4 * jb + 4)
            # dl for chunks 4jb..4jb+3
            rs4 = small.tile([128, 4], FP, tag=f"rs4_{jb}", name=f"rs4_{jb}")
            nc.vector.tensor_reduce(
                rs4[:], rc32[:, 16 * jb:16 * jb + 16].rearrange(
                    "p (i m) -> p i m", m=4), AX.X, AL.add)
            nc.vector.reciprocal(rs4[:], rs4[:])
            nc.vector.tensor_tensor(dl[:, ch4], dg8[:, ch4], rs4[:], AL.mult)
            ptd = jps.tile([4, 128], FP, tag="jt", name=f"ptd_{jb}",
                           padded_shape=[4, 512])
            _tp(nc, ptd[:], dl[:, ch4], ident)
            s4d = small.tile([4, 128], BF, tag=f"s4d_{jb}", name=f"s4d_{jb}")
            nc.vector.tensor_copy(s4d[:], ptd[:])
            dlrow = small.tile([1, MB], BF, tag=f"dlrow_{jb}", name=f"dlrow_{jb}")
            (nc.sync if jb == 0 else nc.scalar).dma_start(
                dlrow[:].rearrange("o (i p) -> o i p", p=128), s4d[:])
            dp = jps.tile([64, MB], FP, tag="jt", name=f"dlsb_{jb}")
            nc.tensor.matmul(dp[:], onesr64, dlrow[:], start=True, stop=True)
            nc.scalar.copy(dls[:, hsl], dp[:])
            # GCN layers for this half
            nc.vector.tensor_tensor(hca[0:64, hsl], ph1sb[:, hsl], dls[:, hsl],
                                    AL.mult)
            ph2 = jps.tile([64, MB], FP, tag="jt", name=f"ph2_{jb}")
            nc.tensor.matmul(ph2[:], w2a, hca[:, hsl], start=True, stop=True)
            nc.vector.tensor_tensor(hcb[0:64, hsl], ph2[:], dls[:, hsl], AL.mult)
            ph3 = jps.tile([64, MB], FP, tag="jt", name=f"ph3_{jb}")
            nc.tensor.matmul(ph3[:], w3a, hcb[:, hsl], start=True, stop=True)
            nc.vector.tensor_tensor(finsq[0:64, hsl], ph3[:], dls[:, hsl], AL.mult)
            nc.vector.tensor_tensor(finsq[0:64, hsl], finsq[0:64, hsl],
                                    x1aug[0:64, hsl], AL.add)
            # fin LN stats (T layout) for this half
            nc.scalar.square(finsq[64:128, hsl], finsq[0:64, hsl])
            fsp = jps.tile([2, MB], FP, tag="jt", name=f"fsp_{jb}")
            nc.tensor.matmul(fsp[:], sel2, finsq[:, hsl], start=True, stop=True)
            (nc.vector.tensor_copy if jb == 0 else nc.scalar.copy)(
                fsum[:, hsl], fsp[:])
            fst0 = small.tile([4, 128], FP, tag=f"fst0_{jb}", name=f"fst0_{jb}")
            (nc.sync if jb == 0 else nc.scalar).dma_start(
                fst0[:], fsum[0:1, hsl].rearrange("o (i p) -> o i p", p=128))
            fst1 = small.tile([4, 128], FP, tag=f"fst1_{jb}", name=f"fst1_{jb}")
            (nc.scalar if jb == 0 else nc.sync).dma_start(
                fst1[:], fsum[1:2, hsl].rearrange("o (i p) -> o i p", p=128))
            _, _, fa, fc = _stat_land(nc, small, fst0[:], fst1[:], 4, f"fl{jb}",
                                      epsc4, 1.0 / 64)
            fap = jps.tile([128, 4], FP, tag="jt", name=f"fap_{jb}",
                           padded_shape=[128, 512])
            _tp(nc, fap[:], fa[:], ident)
            facol = small.tile([128, 4], FP, tag=f"facol_{jb}", name=f"facol_{jb}")
            nc.vector.tensor_copy(facol[:], fap[:])
            fcp = jps.tile([128, 4], FP, tag="jt", name=f"fcp_{jb}",
                           padded_shape=[128, 512])
            _tp(nc, fcp[:], fc[:], ident)
            fccol = small.tile([128, 4], FP, tag=f"fccol_{jb}", name=f"fccol_{jb}")
            nc.scalar.copy(fccol[:], fcp[:])
            for ii in range(4):
                i = 4 * jb + ii
                pt = jps.tile([128, 64], BF, tag="jt", name=f"fin_{i}",
                              padded_shape=[128, 1024])
                _tp(nc, pt[:], finsq[0:64, i * 128:(i + 1) * 128], identb)
                if ii % 2 == 0:
                    nc.vector.tensor_scalar(fin[:, i * 64:(i + 1) * 64], pt[:],
                                            facol[:, ii:ii + 1], fccol[:, ii:ii + 1],
                                            AL.mult, AL.add)
                else:
                    nc.scalar.activation(fin[:, i * 64:(i + 1) * 64], pt[:],
                                         AF.Identity, scale=facol[:, ii:ii + 1],
                                         bias=fccol[:, ii:ii + 1])
            f3 = fin[:, 256 * jb:256 * (jb + 1)].rearrange("p (g e) -> p g e", e=64)
            fg3 = brows["lng"][:].unsqueeze(1).broadcast_to([128, 4, 64])
            fb3 = brows["lnb"][:].unsqueeze(1).broadcast_to([128, 4, 64])
            nc.vector.tensor_tensor(f3, f3, fg3, AL.mult)
            nc.vector.tensor_tensor(f3, f3, fb3, AL.add)
            nc.sync.dma_start(
                io["outH"][512 * jb:512 * (jb + 1), :].rearrange(
                    "(i p) e -> p i e", p=128),
                fin[:, 256 * jb:256 * (jb + 1)].rearrange("p (i e) -> p i e", e=64))


def _build():
    if "nc" in _CACHE:
        return _CACHE["nc"]
    nc = bacc.Bacc("TRN2", target_bir_lowering=False, debug=False,
                   enable_asserts=True, num_devices=8)
    io = {}
    io["xT"] = nc.dram_tensor("xT", [G, N], BF, kind="ExternalInput").ap()
    io["lastT"] = nc.dram_tensor("lastT", [G, N], BF, kind="ExternalInput").ap()
    io["origT"] = nc.dram_tensor("origT", [E, HALF], BF, kind="ExternalInput").ap()
    io["corr4"] = nc.dram_tensor("corr4", [4, N], BF, kind="ExternalInput").ap()
    io["wpack"] = nc.dram_tensor("wpack", [128, WPACK_W], BF, kind="ExternalInput").ap()
    io["fpack"] = nc.dram_tensor("fpack", [128, FPACK_W], FP, kind="ExternalInput").ap()
    io["outH"] = nc.dram_tensor("outH", [HALF, E], FP, kind="ExternalOutput").ap()
    io["lastH"] = nc.dram_tensor("lastH", [HALF, G], FP, kind="ExternalOutput").ap()

    with tile.TileContext(nc) as tc:
        with ExitStack() as ctx:
            _emit(ctx, tc, io)
    nc.compile()
    nc.m = get_hw_module(nc.m)
    _CACHE["nc"] = nc
    return nc


def _host_prep(inputs):
    f32 = np.float32
    bf = ml_dtypes.bfloat16
    inp = {k: np.asarray(v, f32) for k, v in inputs.items()}
    ch = 1.0 + inp["mlp_w"].sum(axis=0)
    assert (ch > 0).all(), "head-mixing scale fold requires positive c_h"
    g, b = inp["bn_g"], inp["bn_b"]
    qw_c = inp["q_w"] * np.repeat(ch / np.sqrt(G), G)[None, :]
    Wq = g[:, None] * qw_c
    qA = np.concatenate([Wq, Wq.sum(axis=0)[None], (b @ qw_c)[None]], axis=0)
    Wk = g[:, None] * inp["k_w"]
    kA = np.concatenate([Wk, Wk.sum(axis=0)[None], (b @ inp["k_w"])[None]], axis=0)
    w1 = inp["gcn_w1"]
    w1a = np.concatenate([w1, -(inp["gcn_b3"] @ w1)[None]], axis=0)
    w2a = np.concatenate([inp["gcn_w2"], (inp["gcn_b1"] @ inp["gcn_w2"])[None]], axis=0)
    w3a = np.concatenate([inp["gcn_w3"], (inp["gcn_b2"] @ inp["gcn_w3"])[None]], axis=0)
    fc3a = np.concatenate([inp["fc3_w"], inp["fc3_b"][None, :]], axis=0)

    wpack = np.zeros((128, WPACK_W), f32)
    wpack[0:128, W_IDB:W_IDB + 128] = np.eye(128)
    wpack[0:128, W_WZ:W_WZ + 64] = inp["w_z"]
    wpack[0:128, W_WR:W_WR + 64] = inp["w_r"]
    wpack[0:128, W_WH:W_WH + 64] = inp["w_h"]
    wpack[0:66, W_QA:W_QA + 256] = qA
    wpack[0:66, W_KA:W_KA + 256] = kA
    wpack[0:64, W_FC1:W_FC1 + 16] = inp["fc1_w"]
    wpack[0:16, W_FC2:W_FC2 + 2] = inp["fc2_w"]
    wpack[0:3, W_FC3A:W_FC3A + 64] = fc3a
    wpack[0:65, W_W1A:W_W1A + 64] = w1a
    wpack[0:65, W_W2A:W_W2A + 64] = w2a
    wpack[0:65, W_W3A:W_W3A + 64] = w3a
    wpack[0:64, W_SEL:W_SEL + 1] = 1.0
    wpack[64:128, W_SEL + 1:W_SEL + 2] = 1.0
    wpack[:, W_ONE:W_ONE + 128] = 1.0

    fpack = np.zeros((128, FPACK_W), f32)
    fpack[0:128, F_IDF:F_IDF + 128] = np.eye(128)
    fpack[0:16, F_B + 0] = inp["fc1_b"]
    fpack[0:2, F_B + 1] = inp["fc2_b"]
    fpack[0:64, F_B + 2] = inp["fc3_b"]
    fpack[0:128, F_EPS] = EPS
    fpack[0:64, F_XG] = inp["x_nom_g"]
    fpack[0:64, F_XB3] = inp["x_nom_b"] + inp["gcn_b3"]
    for k, nm in enumerate(("bn_g", "bn_b", "last_nom_g", "last_nom_b")):
        fpack[0, F_BN + 64 * k:F_BN + 64 * (k + 1)] = inp[nm]

    def c(a, dt=bf):
        return np.ascontiguousarray(np.asarray(a, dt))

    shared = {"wpack": c(wpack), "fpack": c(fpack, f32)}
    in_maps = []
    for core in range(8):
        bi, h = core // 2, core % 2
        off = h * HALF
        corr4 = np.stack([
            np.roll(inp["attn_norm_g"], -off),
            np.roll(inp["skip_norm_g"], -off),
            np.roll(inp["attn_norm_b"] + inp["skip_norm_b"], -off),
            np.ones(N, f32),
        ])
        m = dict(shared)
        m["xT"] = c(np.roll(inp["x"][bi], -off, axis=0).T)
        m["lastT"] = c(np.roll(inp["last_G_emb"][bi], -off, axis=0).T)
        m["origT"] = c(inp["orig_x"][bi, off:off + HALF].T)
        m["corr4"] = c(corr4)
        in_maps.append(m)
    return in_maps


def run(inputs, trace=False):
    nc = _build()
    in_maps = _host_prep(inputs)
    res = run_bass_kernel_spmd(nc, in_maps, core_ids=list(range(8)), trace=trace)
    out = np.zeros((B, N, E), np.float32)
    last = np.zeros((B, N, G), np.float32)
    for core in range(8):
        bi, h = core // 2, core % 2
        off = h * HALF
        out[bi, off:off + HALF] = res.results[core]["outH"]
        last[bi, off:off + HALF] = res.results[core]["lastH"]
    return (out, last), res


def kernel(**inputs):
    return run(inputs)[0]


# revision 17
# speedup vs baseline: 1.0714x; 1.0714x over previous
"""Trainium2 Bass kernel for nn_DGCN (gnn_message_passing).

Sharding: 8 shards = (batch b in 0..3, row-half h in 0..1). Each core gets
the full 2048-node K-side tensors of its batch with the node axis ROTATED
by h*1024 so the adjacency diagonal lands at the same tile position on
every core (uniform SPMD program); the core computes rows 0..1023 of the
rotated order, which are rows [h*1024, (h+1)*1024) of the original order.

v4 — breadth-first emission (engine queues are strict FIFO; depth-first
emission head-of-line blocks every queue), row-layout LN statistics:
 - All heavy tensors bf16; host pre-transposes x/last/orig; small params
   arrive in two packed mega-tiles (one bf16, one fp32) = 2 DMAs.
 - Head-mix scalars c_h and the Hg-LayerNorm gain fold into q_w/k_w; the
   LN shift becomes an extra contraction row of an augmented [66 x 256]
   weight (rhs rows = [Hg*a ; c ; 1]).
 - Per-node LN stats (Hg, xo, att/soc rows) are computed as [1/2, 512]
   PSUM rows by matmuls against ones/selector weights, scattered into
   [8/16, 128] chunk layout by SBUF-SBUF DMA for the pointwise math, and
   scattered back as broadcast rows fed to K=1 matmuls.
 - xo LayerNorm runs in T layout (feature axis on partitions) with the
   gain applied as a per-partition activation scale; no row-major xo.
 - diag(L)_i = relu(bracket)_ii / rowsum_j relu(bracket)_ij is invariant
   to positive per-row scales, so the 1/sdA row scale of the fused
   pre-relu matrix cancels; only the x3 lhsT rows carry rsS/rsA and the
   stat rows carry {-muA, -rho*muS, sdA}.
 - relu row-sums via accum_out; GCN biases ride the next layer's matmul
   as host-folded b@W rows against a constant ones row.
 - Phase I is software-pipelined two chunks deep over all 8 PSUM banks.
"""

import sys

if '/opt/trn_rl_repo' not in sys.path:
    sys.path.insert(0, '/opt/trn_rl_repo')

from contextlib import ExitStack

import numpy as np
import ml_dtypes

import concourse.bass as bass
import concourse.tile as tile
from concourse import bacc, mybir
from concourse.bass_interp import get_hw_module
from concourse.bass_utils import run_bass_kernel_spmd

FP = mybir.dt.float32
BF = mybir.dt.bfloat16
AL = mybir.AluOpType
AF = mybir.ActivationFunctionType
AX = mybir.AxisListType

B, N, E, G, H = 4, 2048, 64, 64, 4
D = H * G          # 256
HALF = N // 2      # own rows per core
NCH = N // 128     # 16 chunks over all nodes
HCH = HALF // 128  # 8 own chunks
MB = 512
NMB = N // MB      # 4
EPS = 1e-5

# wpack (bf16 [128, WPACK_W]) column layout
W_IDB, W_WZ, W_WR, W_WH = 0, 128, 192, 256
W_QA, W_KA = 320, 576
W_FC1, W_FC2, W_FC3A = 832, 848, 850
W_W1A, W_W2A, W_W3A = 914, 978, 1042
W_SEL, W_ONE = 1106, 1108
WPACK_W = 1280
# fpack (fp32 [128, FPACK_W]) column layout
F_IDF, F_B, F_EPS, F_XG, F_XB3, F_BN = 0, 128, 132, 133, 134, 136
FPACK_W = 528

_CACHE = {}


def _tp(nc, out_ap, in_ap, ident):
    k = in_ap.partition_size()
    nc.tensor.transpose(out_ap, in_ap, ident[0:k, 0:k])


def _leaky(nc, dst):
    nc.vector.scalar_tensor_tensor(dst, dst, 0.01, dst, AL.mult, AL.max)


def _ln_rows(nc, pool, t_in, t_out, g_b, b_b, ngr, tag, epsc):
    """LayerNorm over 64-wide groups: t_in [128, ngr*64] -> t_out."""
    a3 = t_in[:].rearrange("p (g e) -> p g e", e=64)
    o3 = t_out[:].rearrange("p (g e) -> p g e", e=64)
    sm = pool.tile([128, ngr], FP, tag=f"{tag}_sm")
    nc.vector.tensor_reduce(sm[:], a3, AX.X, AL.add)
    sq = pool.tile([128, ngr * 64], FP, tag=f"{tag}_sq")
    nc.scalar.square(sq[:], t_in[:])
    sqs = pool.tile([128, ngr], FP, tag=f"{tag}_sqs")
    nc.vector.tensor_reduce(sqs[:], sq[:].rearrange("p (g e) -> p g e", e=64),
                            AX.X, AL.add)
    mu = pool.tile([128, ngr], FP, tag=f"{tag}_mu")
    nc.vector.tensor_scalar(mu[:], sm[:], 1.0 / 64, None, AL.mult)
    mu2 = pool.tile([128, ngr], FP, tag=f"{tag}_mu2")
    nc.vector.tensor_tensor(mu2[:], mu[:], mu[:], AL.mult)
    var = pool.tile([128, ngr], FP, tag=f"{tag}_var")
    nc.vector.scalar_tensor_tensor(var[:], sqs[:], 1.0 / 64, mu2[:], AL.mult,
                                   AL.subtract)
    sd = pool.tile([128, ngr], FP, tag=f"{tag}_sd")
    nc.scalar.activation(sd[:], var[:], AF.Sqrt, bias=epsc)
    rs = pool.tile([128, ngr], FP, tag=f"{tag}_rs")
    nc.vector.reciprocal(rs[:], sd[:])
    mu_b = mu[:].unsqueeze(2).broadcast_to([128, ngr, 64])
    rs_b = rs[:].unsqueeze(2).broadcast_to([128, ngr, 64])
    g3 = g_b.unsqueeze(1).broadcast_to([128, ngr, 64])
    b3 = b_b.unsqueeze(1).broadcast_to([128, ngr, 64])
    xc = pool.tile([128, ngr * 64], FP, tag=f"{tag}_xc")
    xc3 = xc[:].rearrange("p (g e) -> p g e", e=64)
    nc.vector.tensor_tensor(xc3, a3, mu_b, AL.subtract)
    nc.vector.tensor_tensor(xc3, xc3, rs_b, AL.mult)
    nc.vector.tensor_tensor(xc3, xc3, g3, AL.mult)
    nc.vector.tensor_tensor(o3, xc3, b3, AL.add)


def _stat_land(nc, small, sum_t, sq_t, nch, tag, epsc, inv):
    """[nch,128] sums/sumsq -> (a, c) = (1/sd, -mu/sd), both [nch, 128] fp32."""
    mu = small.tile([nch, 128], FP, tag=f"{tag}_mu", name=f"{tag}_mu")
    nc.vector.tensor_scalar(mu[:], sum_t, inv, None, AL.mult)
    mu2 = small.tile([nch, 128], FP, tag=f"{tag}_mu2", name=f"{tag}_mu2")
    nc.vector.tensor_tensor(mu2[:], mu[:], mu[:], AL.mult)
    var = small.tile([nch, 128], FP, tag=f"{tag}_var", name=f"{tag}_var")
    nc.vector.scalar_tensor_tensor(var[:], sq_t, inv, mu2[:], AL.mult, AL.subtract)
    sd = small.tile([nch, 128], FP, tag=f"{tag}_sd", name=f"{tag}_sd")
    nc.scalar.activation(sd[:], var[:], AF.Sqrt, bias=epsc)
    a = small.tile([nch, 128], FP, tag=f"{tag}_a", name=f"{tag}_a")
    nc.vector.reciprocal(a[:], sd[:])
    c = small.tile([nch, 128], FP, tag=f"{tag}_c", name=f"{tag}_c")
    nc.vector.scalar_tensor_tensor(c[:], mu[:], -1.0, a[:], AL.mult, AL.mult)
    return mu, sd, a, c


def _emit(ctx: ExitStack, tc: tile.TileContext, io: dict):
    nc = tc.nc

    persist = ctx.enter_context(tc.tile_pool(name="persist", bufs=1))
    small = ctx.enter_context(tc.tile_pool(name="small", bufs=1))

    # ---------------- packed params (2 DMAs) ----------------
    wp = persist.tile([128, WPACK_W], BF, tag="wp")
    nc.sync.dma_start(wp[:], io["wpack"][:])
    fp_ = persist.tile([128, FPACK_W], FP, tag="fp_")
    nc.scalar.dma_start(fp_[:], io["fpack"][:])

    identb = wp[:, W_IDB:W_IDB + 128]
    wz = wp[:, W_WZ:W_WZ + 64]
    wr = wp[:, W_WR:W_WR + 64]
    wh = wp[:, W_WH:W_WH + 64]
    kA = wp[0:66, W_KA:W_KA + 256]
    fc1s = wp[0:64, W_FC1:W_FC1 + 16]
    fc2s = wp[0:16, W_FC2:W_FC2 + 2]
    fc3s = wp[0:2, W_FC3A:W_FC3A + 64]
    fc3a = wp[0:3, W_FC3A:W_FC3A + 64]
    w1a = wp[0:65, W_W1A:W_W1A + 64]
    w2a = wp[0:65, W_W2A:W_W2A + 64]
    w3a = wp[0:65, W_W3A:W_W3A + 64]
    sel2 = wp[:, W_SEL:W_SEL + 2]
    ones128c = wp[:, W_ONE:W_ONE + 1]
    ones64c = wp[0:64, W_ONE:W_ONE + 1]
    onesr128 = wp[0:1, W_ONE:W_ONE + 128]
    onesr64 = wp[0:1, W_ONE:W_ONE + 64]

    ident = fp_[:, F_IDF:F_IDF + 128]
    fc1b = fp_[0:16, F_B + 0:F_B + 1]
    fc2b = fp_[0:2, F_B + 1:F_B + 2]
    fc3b = fp_[0:64, F_B + 2:F_B + 3]
    epsc128 = fp_[0:128, F_EPS:F_EPS + 1]
    epsc16 = fp_[0:16, F_EPS:F_EPS + 1]
    epsc8 = fp_[0:8, F_EPS:F_EPS + 1]
    xng_c = fp_[0:64, F_XG:F_XG + 1]
    xb3_c = fp_[0:64, F_XB3:F_XB3 + 1]

    # LN parameter rows -> [128, 64] broadcast tiles via gpsimd (small)
    brows = {}
    for k, nm in enumerate(("bng", "bnb", "lng", "lnb")):
        t = persist.tile([128, 64], FP, tag=f"{nm}_b", name=f"{nm}_b")
        nc.gpsimd.partition_broadcast(
            t[:], fp_[0:1, F_BN + 64 * k:F_BN + 64 * (k + 1)])
        brows[nm] = t

    # ---------------- big persistent tiles ----------------
    xT = persist.tile([64, N], BF, tag="xT")
    lastT = persist.tile([64, N], BF, tag="lastT")
    c1 = persist.tile([128, N], BF, tag="c1")      # [x3 ; last]
    c2 = persist.tile([128, N], BF, tag="c2")      # [r*last ; x3]
    hgsq = persist.tile([128, N], BF, tag="hgsq")  # [Hg_raw ; Hg_raw^2]
    HgQ = persist.tile([66, N], BF, tag="HgQ")     # [Hg*a ; c ; 1]
    osq = persist.tile([128, HALF], BF, tag="osq")  # [origT ; origT^2]
    a_row = persist.tile([1, N], BF, tag="a_row")
    kT0 = persist.tile([128, N], BF, tag="kT0")
    kT1 = persist.tile([128, N], BF, tag="kT1")
    k2T0 = persist.tile([128, N], BF, tag="k2T0")
    k2T1 = persist.tile([128, N], BF, tag="k2T1")
    qT0 = persist.tile([128, HALF], BF, tag="qT0")
    qT1 = persist.tile([128, HALF], BF, tag="qT1")
    x3gs = persist.tile([67, N], BF, tag="x3gs")   # [x3*gs ; ga ; gs ; cb]
    x3rA = persist.tile([67, HALF], BF, tag="x3rA")
    ga_b = persist.tile([128, N], BF, tag="ga_b")
    gs_b = persist.tile([64, N], BF, tag="gs_b")
    ga_r = persist.tile([1, N], BF, tag="ga_r")
    gs_r = persist.tile([1, N], BF, tag="gs_r")
    gt_sb = persist.tile([128, 256], BF, tag="gt_sb")
    gb_sb = persist.tile([128, 256], BF, tag="gb_sb")
    gs_f = persist.tile([64, 64], BF, tag="gs_f")
    ks0 = persist.tile([128, 1], BF, tag="ks0")
    ks1 = persist.tile([128, 1], BF, tag="ks1")
    xsb = persist.tile([64, 1], BF, tag="xsb")
    rc32 = persist.tile([128, 4 * HCH], FP, tag="rc32")
    dg8 = persist.tile([128, HCH], FP, tag="dg8")
    dl = persist.tile([128, HCH], FP, tag="dl")
    x1T = persist.tile([16, N], BF, tag="x1T")
    x2a = persist.tile([3, N], BF, tag="x2a")      # [x2 ; 1]
    e0sb = persist.tile([128, HALF], BF, tag="e0sb")
    e1sb = persist.tile([128, HALF], BF, tag="e1sb")
    essb = persist.tile([64, HALF], BF, tag="essb")
    ph1sb = persist.tile([64, HALF], BF, tag="ph1sb")
    finsq = persist.tile([128, HALF], BF, tag="finsq")
    cT_sb = persist.tile([128, NCH], FP, tag="cT_sb")
    lastR = persist.tile([128, HCH * 64], FP, tag="lastR")
    x1aug = persist.tile([65, HALF], BF, tag="x1aug")  # [xo^T + b3 ; 1]
    hca = persist.tile([65, HALF], BF, tag="hca")
    hcb = persist.tile([65, HALF], BF, tag="hcb")
    fin = persist.tile([128, HCH * 64], FP, tag="fin")

    # input loads
    nc.sync.dma_start(xT[:], io["xT"][:])
    nc.sync.dma_start(lastT[:], io["lastT"][:])
    nc.sync.dma_start(c1[64:128, :], io["lastT"][:])
    nc.sync.dma_start(osq[0:64, :], io["origT"][:])
    nc.sync.dma_start(x3gs[64:67, :], io["corr4"][0:3, :])
    nc.sync.dma_start(ga_r[:], io["corr4"][0:1, :])
    nc.sync.dma_start(gs_r[:], io["corr4"][1:2, :])
    # constant-ones rows
    nc.gpsimd.dma_start(HgQ[65:66, :], io["corr4"][3:4, :])
    nc.gpsimd.dma_start(x2a[2:3, :], io["corr4"][3:4, :])
    nc.gpsimd.dma_start(x1aug[64:65, :], io["corr4"][3:4, 0:HALF])
    nc.gpsimd.dma_start(hca[64:65, :], io["corr4"][3:4, 0:HALF])
    nc.gpsimd.dma_start(hcb[64:65, :], io["corr4"][3:4, 0:HALF])

    frontA = ExitStack()
    fps = frontA.enter_context(tc.tile_pool(name="fps", bufs=6, space="PSUM"))
    gw = frontA.enter_context(tc.tile_pool(name="gw", bufs=4))

    MBs = [slice(j * MB, (j + 1) * MB) for j in range(NMB)]
    HBs = [slice(j * MB, (j + 1) * MB) for j in range(2)]

    # ---- ga / gs broadcast tiles via K=1 matmuls (breadth) ----
    gps_ = [fps.tile([128, MB], FP, tag="fp", name=f"gab_{j}") for j in range(NMB)]
    gss_ = [fps.tile([64, MB], FP, tag="fp", name=f"gsb_{j}") for j in range(2)]
    for j in range(NMB):
        nc.tensor.matmul(gps_[j][:], onesr128, ga_r[:, MBs[j]], start=True, stop=True)
    for j in range(2):
        nc.tensor.matmul(gss_[j][:], onesr64, gs_r[:, j * MB:(j + 1) * MB],
                         start=True, stop=True)
    for j in range(NMB):
        (nc.vector.tensor_copy if j % 2 == 0 else nc.scalar.copy)(
            ga_b[:, MBs[j]], gps_[j][:])
    for j in range(2):
        (nc.scalar.copy if j % 2 == 0 else nc.vector.tensor_copy)(
            gs_b[:, slice(j * MB, (j + 1) * MB)], gss_[j][:])
    gss2_ = [fps.tile([64, MB], FP, tag="fp", name=f"gsb2_{j}") for j in range(2)]
    for j in range(2):
        sl = slice((2 + j) * MB, (3 + j) * MB)
        nc.tensor.matmul(gss2_[j][:], onesr64, gs_r[:, sl], start=True, stop=True)
        (nc.vector.tensor_copy if j % 2 == 0 else nc.scalar.copy)(
            gs_b[:, sl], gss2_[j][:])

    # ---- xo stats (input-only dependent, fills the early pipeline) ----
    nc.scalar.square(osq[64:128, :], osq[0:64, :])
    oxp = [fps.tile([2, MB], FP, tag="fp", name=f"oxp_{j}") for j in range(2)]
    for j in range(2):
        nc.tensor.matmul(oxp[j][:], sel2, osq[:, HBs[j]], start=True, stop=True)
    oxs = small.tile([2, HALF], FP, tag="oxs")
    for j in range(2):
        (nc.vector.tensor_copy if j == 0 else nc.scalar.copy)(oxs[:, HBs[j]], oxp[j][:])
    oxs0 = small.tile([HCH, 128], FP, tag="oxs0")
    nc.sync.dma_start(oxs0[:], oxs[0:1, :].rearrange("o (i p) -> o i p", p=128))
    oxs1 = small.tile([HCH, 128], FP, tag="oxs1")
    nc.sync.dma_start(oxs1[:], oxs[1:2, :].rearrange("o (i p) -> o i p", p=128))
    _, _, oa, oc = _stat_land(nc, small, oxs0[:], oxs1[:], HCH, "ox", epsc8, 1.0 / 64)
    oa8 = small.tile([HCH, 128], BF, tag="oa8")
    nc.vector.tensor_copy(oa8[:], oa[:])
    oc8 = small.tile([HCH, 128], BF, tag="oc8")
    nc.scalar.copy(oc8[:], oc[:])
    oar = small.tile([1, HALF], BF, tag="oar")
    nc.sync.dma_start(oar[:].rearrange("o (i p) -> o i p", p=128), oa8[:])
    ocr = small.tile([1, HALF], BF, tag="ocr")
    nc.sync.dma_start(ocr[:].rearrange("o (i p) -> o i p", p=128), oc8[:])

    # ============ hyper fc stack (breadth-first stages) ============
    xacc = small.tile([64, NMB], FP, tag="xacc")
    p1 = [fps.tile([16, MB], FP, tag="fp", name=f"p1_{j}") for j in range(NMB)]
    for j in range(NMB):
        nc.tensor.matmul(p1[j][:], fc1s, xT[:, MBs[j]], start=True, stop=True)
    for j in range(NMB):
        nc.scalar.activation(x1T[:, MBs[j]], p1[j][:], AF.Sigmoid, bias=fc1b)
    p2 = [fps.tile([2, MB], FP, tag="fp", name=f"p2_{j}") for j in range(NMB)]
    for j in range(NMB):
        nc.tensor.matmul(p2[j][:], fc2s, x1T[:, MBs[j]], start=True, stop=True)
    for j in range(NMB):
        nc.scalar.activation(x2a[0:2, MBs[j]], p2[j][:], AF.Sigmoid, bias=fc2b)
    p3 = [fps.tile([64, MB], FP, tag="fp", name=f"p3_{j}") for j in range(NMB)]
    for j in range(NMB):
        nc.tensor.matmul(p3[j][:], fc3s, x2a[0:2, MBs[j]], start=True, stop=True)
    for j in range(NMB):
        nc.scalar.activation(c1[0:64, MBs[j]], p3[j][:], AF.Identity, bias=fc3b,
                             accum_out=xacc[:, j:j + 1])
    for j in range(NMB):
        nc.vector.tensor_copy(c2[64:128, MBs[j]], c1[0:64, MBs[j]])
    xs_f = small.tile([64, 1], FP, tag="xs_f")
    nc.vector.tensor_reduce(xs_f[:], xacc[:], AX.X, AL.add)
    nc.vector.tensor_copy(xsb[:], xs_f[:])

    # ================= GRU gates (breadth-first stages) =================
    zp = [fps.tile([64, MB], FP, tag="fp", name=f"zp_{j}") for j in range(NMB)]
    for j in range(NMB):
        nc.tensor.matmul(zp[j][:], wz, c1[:, MBs[j]], start=True, stop=True)
    zt = [gw.tile([64, MB], BF, tag="zt", name=f"zt_{j}") for j in range(NMB)]
    for j in range(NMB):
        nc.scalar.activation(zt[j][:], zp[j][:], AF.Sigmoid)
    rp = [fps.tile([64, MB], FP, tag="fp", name=f"rp_{j}") for j in range(NMB)]
    for j in range(NMB):
        nc.tensor.matmul(rp[j][:], wr, c1[:, MBs[j]], start=True, stop=True)
    rt = [gw.tile([64, MB], BF, tag="rt", name=f"rt_{j}") for j in range(NMB)]
    for j in range(NMB):
        nc.scalar.activation(rt[j][:], rp[j][:], AF.Sigmoid)
    for j in range(NMB):
        nc.vector.tensor_tensor(c2[0:64, MBs[j]], rt[j][:], lastT[:, MBs[j]], AL.mult)
    hp = [fps.tile([64, MB], FP, tag="fp", name=f"hp_{j}") for j in range(NMB)]
    for j in range(NMB):
        nc.tensor.matmul(hp[j][:], wh, c2[:, MBs[j]], start=True, stop=True)
    ht = [gw.tile([64, MB], BF, tag="ht", name=f"ht_{j}") for j in range(NMB)]
    for j in range(NMB):
        nc.scalar.activation(ht[j][:], hp[j][:], AF.Tanh)
    dt_ = [gw.tile([64, MB], BF, tag="dt", name=f"dt_{j}") for j in range(NMB)]
    for j in range(NMB):
        nc.vector.tensor_tensor(dt_[j][:], ht[j][:], lastT[:, MBs[j]], AL.subtract)
    for j in range(NMB):
        nc.vector.tensor_tensor(dt_[j][:], dt_[j][:], zt[j][:], AL.mult)
    for j in range(NMB):
        nc.vector.tensor_tensor(hgsq[0:64, MBs[j]], dt_[j][:], lastT[:, MBs[j]], AL.add)

    # Hg^2 (scalar) then Hg LN stats rows
    nc.scalar.square(hgsq[64:128, 0:HALF], hgsq[0:64, 0:HALF])
    nc.vector.tensor_tensor(hgsq[64:128, HALF:N], hgsq[0:64, HALF:N],
                            hgsq[0:64, HALF:N], AL.mult)
    hsp = [fps.tile([2, MB], FP, tag="fp", name=f"hsp_{j}") for j in range(NMB)]
    for j in range(NMB):
        nc.tensor.matmul(hsp[j][:], sel2, hgsq[:, MBs[j]], start=True, stop=True)
    hsum = small.tile([2, N], FP, tag="hsum")
    for j in range(NMB):
        (nc.vector.tensor_copy if j % 2 == 0 else nc.scalar.copy)(
            hsum[:, MBs[j]], hsp[j][:])
    hst0 = small.tile([NCH, 128], FP, tag="hst0")
    nc.sync.dma_start(hst0[:], hsum[0:1, :].rearrange("o (i p) -> o i p", p=128))
    hst1 = small.tile([NCH, 128], FP, tag="hst1")
    nc.sync.dma_start(hst1[:], hsum[1:2, :].rearrange("o (i p) -> o i p", p=128))
    _, _, ha, hc = _stat_land(nc, small, hst0[:], hst1[:], NCH, "hg", epsc16, 1.0 / 64)
    ha16 = small.tile([NCH, 128], BF, tag="ha16")
    nc.vector.tensor_copy(ha16[:], ha[:])
    hc16 = small.tile([NCH, 128], BF, tag="hc16")
    nc.scalar.copy(hc16[:], hc[:])
    nc.sync.dma_start(a_row[:].rearrange("o (i p) -> o i p", p=128), ha16[:])
    nc.sync.dma_start(HgQ[64:65, :].rearrange("o (i p) -> o i p", p=128), hc16[:])
    # c in chunk-column layout for the lastH bias path
    pcc = fps.tile([128, NCH], FP, tag="fp", name="pcc", padded_shape=[128, 512])
    _tp(nc, pcc[:], hc[:], ident)
    nc.scalar.copy(cT_sb[:], pcc[:])

    # HgA = Hg_raw * a (K=1 broadcast matmul + fused multiply from PSUM)
    ab = [fps.tile([64, MB], FP, tag="fp", name=f"ab_{j}") for j in range(NMB)]
    for j in range(NMB):
        nc.tensor.matmul(ab[j][:], onesr64, a_row[:, MBs[j]], start=True, stop=True)
    for j in range(NMB):
        nc.vector.tensor_tensor(HgQ[0:64, MBs[j]], hgsq[0:64, MBs[j]], ab[j][:],
                                AL.mult)

    # ===================== q / k projections =====================
    kacc = small.tile([128, 8], FP, tag="kacc")
    kjobs = []
    for half, dst in ((0, kT0), (1, kT1)):
        for j in range(NMB):
            kjobs.append((dst, slice(W_KA + 128 * half, W_KA + 128 * (half + 1)),
                          MBs[j], kacc[:, 4 * half + j:4 * half + j + 1]))
    qjobs = []
    for half, dst in ((0, qT0), (1, qT1)):
        for j in range(2):
            qjobs.append((dst, slice(W_QA + 128 * half, W_QA + 128 * (half + 1)),
                          HBs[j], None))
    kq_ps = []
    for idx, (dst, wsl, sl, acc) in enumerate(kjobs + qjobs):
        kp = fps.tile([128, MB], FP, tag="fp", name=f"kqp_{idx}")
        nc.tensor.matmul(kp[:], wp[0:66, wsl], HgQ[:, sl], start=True, stop=True)
        kq_ps.append(kp)
    for idx, (dst, wsl, sl, acc) in enumerate(kjobs + qjobs):
        nc.scalar.copy(dst[:, sl], kq_ps[idx][:])
        if acc is not None:
            nc.vector.scalar_tensor_tensor(dst[:, sl], dst[:, sl], 0.01, dst[:, sl],
                                           AL.mult, AL.max, accum_out=acc)
        else:
            _leaky(nc, dst[:, sl])
    ks_f = small.tile([128, 2], FP, tag="ks_f")
    nc.vector.tensor_reduce(ks_f[:], kacc[:].rearrange("p (h j) -> p h j", j=4),
                            AX.X, AL.add)
    nc.vector.tensor_copy(ks0[:], ks_f[:, 0:1])
    nc.vector.tensor_copy(ks1[:], ks_f[:, 1:2])
    # k2 = k * ga ; x3gs rows 0:64 = x3 * gs
    nc.vector.tensor_tensor(k2T0[:], kT0[:], ga_b[:], AL.mult)
    nc.vector.tensor_tensor(k2T1[:], kT1[:], ga_b[:], AL.mult)
    nc.vector.tensor_tensor(x3gs[0:64, :], c1[0:64, :], gs_b[:], AL.mult)

    frontA.close()

    # ===================== Gram matrices =====================
    with tc.tile_pool(name="gpsp", bufs=3, space="PSUM") as gpsp, \
         tc.tile_pool(name="krpp", bufs=3, space="PSUM") as krpp, \
         tc.tile_pool(name="krp", bufs=3) as krp:
        gt_ps = gpsp.tile([128, 256], FP, tag="g", padded_shape=[128, 512])
        gb_ps = gpsp.tile([128, 256], FP, tag="g", padded_shape=[128, 512])
        for mi in range(NCH):
            msl = slice(mi * 128, (mi + 1) * 128)
            krq = krpp.tile([128, 256], FP, tag="kr", padded_shape=[128, 512])
            nc.tensor.matmul(krq[:], HgQ[:, msl], kA, start=True, stop=True)
            kr = krp.tile([128, 256], BF, tag="kr")
            nc.scalar.copy(kr[:], krq[:])
            _leaky(nc, kr[:])
            nc.tensor.matmul(gt_ps[:], kr[:, 0:128], kr[:],
                             start=(mi == 0), stop=(mi == NCH - 1))
            nc.tensor.matmul(gb_ps[:], kr[:, 128:256], kr[:],
                             start=(mi == 0), stop=(mi == NCH - 1))
        nc.vector.tensor_copy(gt_sb[:], gt_ps[:])
        nc.scalar.copy(gb_sb[:], gb_ps[:])
        gs_ps = gpsp.tile([64, 64], FP, tag="g", padded_shape=[64, 512])
        for mi in range(NCH):
            msl = slice(mi * 128, (mi + 1) * 128)
            xrq = krpp.tile([128, 64], FP, tag="kr", padded_shape=[128, 512])
            nc.tensor.matmul(xrq[:], x2a[:, msl], fc3a, start=True, stop=True)
            xr = krp.tile([128, 64], BF, tag="xr")
            (nc.vector.tensor_copy if mi % 2 == 0 else nc.scalar.copy)(xr[:], xrq[:])
            nc.tensor.matmul(gs_ps[:], xr[:], xr[:],
                             start=(mi == 0), stop=(mi == NCH - 1))
        nc.vector.tensor_copy(gs_f[:], gs_ps[:])

    # ============== own-row stats: S1, T1, S2, T2 rows ==============
    statq = ExitStack()
    ups = statq.enter_context(tc.tile_pool(name="ups", bufs=2, space="PSUM"))
    sps = statq.enter_context(tc.tile_pool(name="sps", bufs=2, space="PSUM"))
    lps = statq.enter_context(tc.tile_pool(name="lps", bufs=2, space="PSUM"))
    # u = G q per 512-half; e = u * q  (separate e0/e1, summed in PSUM below)
    for jb in range(2):
        sl = HBs[jb]
        ut0 = ups.tile([128, MB], FP, tag="ut", name=f"ut0_{jb}")
        nc.tensor.matmul(ut0[:], gt_sb[:, 0:128], qT0[:, sl], start=True, stop=False)
        nc.tensor.matmul(ut0[:], gb_sb[:, 0:128], qT1[:, sl], start=False, stop=True)
        ut1 = ups.tile([128, MB], FP, tag="ut", name=f"ut1_{jb}")
        nc.tensor.matmul(ut1[:], gt_sb[:, 128:256], qT0[:, sl], start=True, stop=False)
        nc.tensor.matmul(ut1[:], gb_sb[:, 128:256], qT1[:, sl], start=False, stop=True)
        nc.vector.tensor_tensor(e0sb[:, sl], ut0[:], qT0[:, sl], AL.mult)
        nc.vector.tensor_tensor(e1sb[:, sl], ut1[:], qT1[:, sl], AL.mult)
    for jb in range(2):
        sl = HBs[jb]
        us = ups.tile([64, MB], FP, tag="ut", name=f"us_{jb}")
        nc.tensor.matmul(us[:], gs_f[:], c1[0:64, sl], start=True, stop=True)
        nc.vector.tensor_tensor(essb[:, sl], us[:], c1[0:64, sl], AL.mult)

    s1sb = small.tile([1, HALF], FP, tag="s1sb")
    t1sb = small.tile([1, HALF], FP, tag="t1sb")
    s2sb = small.tile([1, HALF], FP, tag="s2sb")
    t2sb = small.tile([1, HALF], FP, tag="t2sb")
    for jb in range(2):
        sl = HBs[jb]
        s1p = sps.tile([1, MB], FP, tag="st", name=f"s1p_{jb}", padded_shape=[1, 512])
        nc.tensor.matmul(s1p[:], ks0[:], qT0[:, sl], start=True, stop=False)
        nc.tensor.matmul(s1p[:], ks1[:], qT1[:, sl], start=False, stop=True)
        t1p = sps.tile([1, MB], FP, tag="st", name=f"t1p_{jb}", padded_shape=[1, 512])
        nc.tensor.matmul(t1p[:], xsb[:], c1[0:64, sl], start=True, stop=True)
        nc.scalar.copy(s1sb[:, sl], s1p[:])
        nc.vector.tensor_copy(t1sb[:, sl], t1p[:])
    for jb in range(2):
        sl = HBs[jb]
        s2p = sps.tile([1, MB], FP, tag="st", name=f"s2p_{jb}", padded_shape=[1, 512])
        nc.tensor.matmul(s2p[:], ones128c, e0sb[:, sl], start=True, stop=False)
        nc.tensor.matmul(s2p[:], ones128c, e1sb[:, sl], start=False, stop=True)
        t2p = sps.tile([1, MB], FP, tag="st", name=f"t2p_{jb}", padded_shape=[1, 512])
        nc.tensor.matmul(t2p[:], ones64c, essb[:, sl], start=True, stop=True)
        nc.scalar.copy(s2sb[:, sl], s2p[:])
        nc.vector.tensor_copy(t2sb[:, sl], t2p[:])
    s1t = small.tile([HCH, 128], FP, tag="s1t")
    nc.sync.dma_start(s1t[:], s1sb[:].rearrange("o (i p) -> o i p", p=128))
    t1t = small.tile([HCH, 128], FP, tag="t1t")
    nc.scalar.dma_start(t1t[:], t1sb[:].rearrange("o (i p) -> o i p", p=128))
    s2t = small.tile([HCH, 128], FP, tag="s2t")
    nc.sync.dma_start(s2t[:], s2sb[:].rearrange("o (i p) -> o i p", p=128))
    t2t = small.tile([HCH, 128], FP, tag="t2t")
    nc.scalar.dma_start(t2t[:], t2sb[:].rearrange("o (i p) -> o i p", p=128))

    # ---- fillers for the stats-land latency ----
    # lastH output (Hg LN rows, own half)
    for i in range(HCH):
        pt = lps.tile([128, 64], BF, tag="lpt", name=f"lpt_{i}",
                      padded_shape=[128, 1024])
        _tp(nc, pt[:], HgQ[0:64, i * 128:(i + 1) * 128], identb)
        nc.scalar.activation(lastR[:, i * 64:(i + 1) * 64], pt[:], AF.Identity,
                             bias=cT_sb[:, i:i + 1])
    l3 = lastR[:].rearrange("p (g e) -> p g e", e=64)
    lg3 = brows["bng"][:].unsqueeze(1).broadcast_to([128, HCH, 64])
    lb3 = brows["bnb"][:].unsqueeze(1).broadcast_to([128, HCH, 64])
    nc.vector.tensor_tensor(l3, l3, lg3, AL.mult)
    nc.vector.tensor_tensor(l3, l3, lb3, AL.add)
    nc.sync.dma_start(io["lastH"].rearrange("(i p) e -> p i e", p=128),
                      lastR[:].rearrange("p (i e) -> p i e", e=64))

    # xo affine into x1aug
    oab = [sps.tile([64, MB], FP, tag="st", name=f"oab_{j}") for j in range(2)]
    for j in range(2):
        nc.tensor.matmul(oab[j][:], onesr64, oar[:, HBs[j]], start=True, stop=True)
    ocb = [sps.tile([64, MB], FP, tag="st", name=f"ocb_{j}") for j in range(2)]
    for j in range(2):
        nc.tensor.matmul(ocb[j][:], onesr64, ocr[:, HBs[j]], start=True, stop=True)
    for j in range(2):
        tb = small.tile([64, MB], BF, tag=f"oxt_{j}", name=f"oxt_{j}")
        nc.vector.tensor_tensor(tb[:], osq[0:64, HBs[j]], oab[j][:], AL.mult)
        nc.vector.tensor_tensor(tb[:], tb[:], ocb[j][:], AL.add)
        nc.scalar.activation(x1aug[0:64, HBs[j]], tb[:], AF.Identity,
                             scale=xng_c, bias=xb3_c)

    # GCN layer-1 matmul (dl-independent)
    for jb in range(2):
        ph1 = sps.tile([64, MB], FP, tag="st", name=f"ph1_{jb}")
        nc.tensor.matmul(ph1[:], w1a, x1aug[:, HBs[jb]], start=True, stop=True)
        (nc.scalar.copy if jb == 0 else nc.vector.tensor_copy)(
            ph1sb[:, HBs[jb]], ph1[:])

    # ---- own stats land [8, 128] ----
    muA, sdA, rsA_, _cA = _stat_land(nc, small, s1t[:], s2t[:], HCH, "sa",
                                     epsc8, 1.0 / N)
    muS, sdS, rsS_, _cS = _stat_land(nc, small, t1t[:], t2t[:], HCH, "ss",
                                     epsc8, 1.0 / N)
    rho = small.tile([HCH, 128], FP, tag="rho")
    nc.vector.tensor_tensor(rho[:], rsS_[:], sdA[:], AL.mult)
    r64t = small.tile([HCH, 128], BF, tag="r64t")
    nc.vector.tensor_scalar(r64t[:], muA[:], -1.0, None, AL.mult)
    r65t = small.tile([HCH, 128], BF, tag="r65t")
    nc.vector.scalar_tensor_tensor(r65t[:], muS[:], -1.0, rho[:], AL.mult, AL.mult)
    r66t = small.tile([HCH, 128], BF, tag="r66t")
    nc.scalar.copy(r66t[:], sdA[:])
    rho16 = small.tile([HCH, 128], BF, tag="rho16")
    nc.scalar.copy(rho16[:], rho[:])
    nc.sync.dma_start(x3rA[64:65, :].rearrange("o (i p) -> o i p", p=128), r64t[:])
    nc.scalar.dma_start(x3rA[65:66, :].rearrange("o (i p) -> o i p", p=128), r65t[:])
    nc.sync.dma_start(x3rA[66:67, :].rearrange("o (i p) -> o i p", p=128), r66t[:])
    rho_row = small.tile([1, HALF], BF, tag="rho_row")
    nc.scalar.dma_start(rho_row[:].rearrange("o (i p) -> o i p", p=128), rho16[:])
    for jb in range(2):
        rp_ = sps.tile([64, MB], FP, tag="st", name=f"rhob_{jb}")
        nc.tensor.matmul(rp_[:], onesr64, rho_row[:, HBs[jb]], start=True, stop=True)
        nc.vector.tensor_tensor(x3rA[0:64, HBs[jb]], c1[0:64, HBs[jb]], rp_[:],
                                AL.mult)
    statq.close()

    # =================== phase I: fused adjacency ===================
    with tc.tile_pool(name="zps", bufs=8, space="PSUM") as zps, \
         tc.tile_pool(name="scrv", bufs=2) as scrv, \
         tc.tile_pool(name="scra", bufs=2) as scra:
        ztiles = {}

        def passes12(i):
            csl = slice(i * 128, (i + 1) * 128)
            zpt = [zps.tile([128, MB], FP, tag="zpt", name=f"zp_{i}_{m}")
                   for m in range(NMB)]
            ztiles[i] = zpt
            for mb in range(NMB):
                nc.tensor.matmul(zpt[mb][:], qT0[:, csl],
                                 k2T0[:, mb * MB:(mb + 1) * MB],
                                 start=True, stop=False)
            for mb in range(NMB):
                nc.tensor.matmul(zpt[mb][:], qT1[:, csl],
                                 k2T1[:, mb * MB:(mb + 1) * MB],
                                 start=False, stop=False)

        def pass3(i):
            csl = slice(i * 128, (i + 1) * 128)
            zpt = ztiles[i]
            for mb in range(NMB):
                nc.tensor.matmul(zpt[mb][:], x3rA[:, csl],
                                 x3gs[:, mb * MB:(mb + 1) * MB],
                                 start=False, stop=True)
            for mb in range(NMB):
                acc = rc32[:, 4 * i + mb:4 * i + mb + 1]
                if mb % 2 == 0:
                    scr = scrv.tile([128, MB], FP, tag="scr", name=f"scr_{i}_{mb}")
                    nc.vector.tensor_scalar(scr[:], zpt[mb][:], 0.0, None, AL.max,
                                            AL.add, accum_out=acc)
                else:
                    scr = scra.tile([128, MB], FP, tag="scr2", name=f"scr2_{i}_{mb}")
                    nc.scalar.activation(scr[:], zpt[mb][:], AF.Relu, accum_out=acc)
                if mb == i // 4:
                    off = (i * 128) % MB
                    dsel = scrv.tile([128, 128], FP, tag="dsel", name=f"dsel_{i}")
                    nc.gpsimd.affine_select(
                        out=dsel[:], in_=scr[:, off:off + 128],
                        compare_op=AL.is_equal, fill=0.0, base=0,
                        pattern=[[-1, 128]], channel_multiplier=1)
                    nc.vector.tensor_reduce(dg8[:, i:i + 1], dsel[:], AX.X, AL.add)

        passes12(0)
        passes12(1)
        for i in range(HCH):
            pass3(i)
            if i + 2 < HCH:
                passes12(i + 2)

    # ============== per-half tail: dl -> GCN -> fin LN ==============
    epsc4 = fp_[0:4, F_EPS:F_EPS + 1]
    with tc.tile_pool(name="jps", bufs=4, space="PSUM") as jps:
        dls = small.tile([64, HALF], BF, tag="dls")
        fsum = small.tile([2, HALF], FP, tag="fsum")
        for jb in range(2):
            hsl = HBs[jb]
            ch4 = slice(4 * jb, 4 * jb + 4)
            # dl for chunks 4jb..4jb+3
            rs4 = small.tile([128, 4], FP, tag=f"rs4_{jb}", name=f"rs4_{jb}")
            nc.vector.tensor_reduce(
                rs4[:], rc32[:, 16 * jb:16 * jb + 16].rearrange(
                    "p (i m) -> p i m", m=4), AX.X, AL.add)
            nc.vector.reciprocal(rs4[:], rs4[:])
            nc.vector.tensor_tensor(dl[:, ch4], dg8[:, ch4], rs4[:], AL.mult)
            ptd = jps.tile([4, 128], FP, tag="jt", name=f"ptd_{jb}",
                           padded_shape=[4, 512])
            _tp(nc, ptd[:], dl[:, ch4], ident)
            s4d = small.tile([4, 128], BF, tag=f"s4d_{jb}", name=f"s4d_{jb}")
            nc.vector.tensor_copy(s4d[:], ptd[:])
            dlrow = small.tile([1, MB], BF, tag=f"dlrow_{jb}", name=f"dlrow_{jb}")
            (nc.sync if jb == 0 else nc.scalar).dma_start(
                dlrow[:].rearrange("o (i p) -> o i p", p=128), s4d[:])
            dp = jps.tile([64, MB], FP, tag="jt", name=f"dlsb_{jb}")
            nc.tensor.matmul(dp[:], onesr64, dlrow[:], start=True, stop=True)
            nc.scalar.copy(dls[:, hsl], dp[:])
            # GCN layers for this half
            nc.vector.tensor_tensor(hca[0:64, hsl], ph1sb[:, hsl], dls[:, hsl],
                                    AL.mult)
            ph2 = jps.tile([64, MB], FP, tag="jt", name=f"ph2_{jb}")
            nc.tensor.matmul(ph2[:], w2a, hca[:, hsl], start=True, stop=True)
            nc.vector.tensor_tensor(hcb[0:64, hsl], ph2[:], dls[:, hsl], AL.mult)
            ph3 = jps.tile([64, MB], FP, tag="jt", name=f"ph3_{jb}")
            nc.tensor.matmul(ph3[:], w3a, hcb[:, hsl], start=True, stop=True)
            nc.vector.tensor_tensor(finsq[0:64, hsl], ph3[:], dls[:, hsl], AL.mult)
            nc.vector.tensor_tensor(finsq[0:64, hsl], finsq[0:64, hsl],
                                    x1aug[0:64, hsl], AL.add)
            # fin LN stats (T layout) for this half
            nc.scalar.square(finsq[64:128, hsl], finsq[0:64, hsl])
            fsp = jps.tile([2, MB], FP, tag="jt", name=f"fsp_{jb}")
            nc.tensor.matmul(fsp[:], sel2, finsq[:, hsl], start=True, stop=True)
            (nc.vector.tensor_copy if jb == 0 else nc.scalar.copy)(
                fsum[:, hsl], fsp[:])
            fst0 = small.tile([4, 128], FP, tag=f"fst0_{jb}", name=f"fst0_{jb}")
            (nc.sync if jb == 0 else nc.scalar).dma_start(
                fst0[:], fsum[0:1, hsl].rearrange("o (i p) -> o i p", p=128))
            fst1 = small.tile([4, 128], FP, tag=f"fst1_{jb}", name=f"fst1_{jb}")
            (nc.scalar if jb == 0 else nc.sync).dma_start(
                fst1[:], fsum[1:2, hsl].rearrange("o (i p) -> o i p", p=128))
            _, _, fa, fc = _stat_land(nc, small, fst0[:], fst1[:], 4, f"fl{jb}",
                                      epsc4, 1.0 / 64)
            fap = jps.tile([128, 4], FP, tag="jt", name=f"fap_{jb}",
                           padded_shape=[128, 512])
            _tp(nc, fap[:], fa[:], ident)
            facol = small.tile([128, 4], FP, tag=f"facol_{jb}", name=f"facol_{jb}")
            nc.vector.tensor_copy(facol[:], fap[:])
            fcp = jps.tile([128, 4], FP, tag="jt", name=f"fcp_{jb}",
                           padded_shape=[128, 512])
            _tp(nc, fcp[:], fc[:], ident)
            fccol = small.tile([128, 4], FP, tag=f"fccol_{jb}", name=f"fccol_{jb}")
            nc.scalar.copy(fccol[:], fcp[:])
            for ii in range(4):
                i = 4 * jb + ii
                pt = jps.tile([128, 64], BF, tag="jt", name=f"fin_{i}",
                              padded_shape=[128, 1024])
                _tp(nc, pt[:], finsq[0:64, i * 128:(i + 1) * 128], identb)
                if ii % 2 == 0:
                    nc.vector.tensor_scalar(fin[:, i * 64:(i + 1) * 64], pt[:],
                                            facol[:, ii:ii + 1], fccol[:, ii:ii + 1],
                                            AL.mult, AL.add)
                else:
                    nc.scalar.activation(fin[:, i * 64:(i + 1) * 64], pt[:],
                                         AF.Identity, scale=facol[:, ii:ii + 1],
                                         bias=fccol[:, ii:ii + 1])
            f3 = fin[:, 256 * jb:256 * (jb + 1)].rearrange("p (g e) -> p g e", e=64)
            fg3 = brows["lng"][:].unsqueeze(1).broadcast_to([128, 4, 64])
            fb3 = brows["lnb"][:].unsqueeze(1).broadcast_to([128, 4, 64])
            nc.vector.tensor_tensor(f3, f3, fg3, AL.mult)
            nc.vector.tensor_tensor(f3, f3, fb3, AL.add)
            nc.sync.dma_start(
                io["outH"][512 * jb:512 * (jb + 1), :].rearrange(
                    "(i p) e -> p i e", p=128),
                fin[:, 256 * jb:256 * (jb + 1)].rearrange("p (i e) -> p i e", e=64))


def _build():
    if "nc" in _CACHE:
        return _CACHE["nc"]
    nc = bacc.Bacc("TRN2", target_bir_lowering=False, debug=False,
                   enable_asserts=True, num_devices=8)
    io = {}
    io["xT"] = nc.dram_tensor("xT", [G, N], BF, kind="ExternalInput").ap()
    io["lastT"] = nc.dram_tensor("lastT", [G, N], BF, kind="ExternalInput").ap()
    io["origT"] = nc.dram_tensor("origT", [E, HALF], BF, kind="ExternalInput").ap()
    io["corr4"] = nc.dram_tensor("corr4", [4, N], BF, kind="ExternalInput").ap()
    io["wpack"] = nc.dram_tensor("wpack", [128, WPACK_W], BF, kind="ExternalInput").ap()
    io["fpack"] = nc.dram_tensor("fpack", [128, FPACK_W], FP, kind="ExternalInput").ap()
    io["outH"] = nc.dram_tensor("outH", [HALF, E], FP, kind="ExternalOutput").ap()
    io["lastH"] = nc.dram_tensor("lastH", [HALF, G], FP, kind="ExternalOutput").ap()

    with tile.TileContext(nc) as tc:
        with ExitStack() as ctx:
            _emit(ctx, tc, io)
    nc.compile()
    nc.m = get_hw_module(nc.m)
    _CACHE["nc"] = nc
    return nc


def _host_prep(inputs):
    f32 = np.float32
    bf = ml_dtypes.bfloat16
    inp = {k: np.asarray(v, f32) for k, v in inputs.items()}
    ch = 1.0 + inp["mlp_w"].sum(axis=0)
    assert (ch > 0).all(), "head-mixing scale fold requires positive c_h"
    g, b = inp["bn_g"], inp["bn_b"]
    qw_c = inp["q_w"] * np.repeat(ch / np.sqrt(G), G)[None, :]
    Wq = g[:, None] * qw_c
    qA = np.concatenate([Wq, Wq.sum(axis=0)[None], (b @ qw_c)[None]], axis=0)
    Wk = g[:, None] * inp["k_w"]
    kA = np.concatenate([Wk, Wk.sum(axis=0)[None], (b @ inp["k_w"])[None]], axis=0)
    w1 = inp["gcn_w1"]
    w1a = np.concatenate([w1, -(inp["gcn_b3"] @ w1)[None]], axis=0)
    w2a = np.concatenate([inp["gcn_w2"], (inp["gcn_b1"] @ inp["gcn_w2"])[None]], axis=0)
    w3a = np.concatenate([inp["gcn_w3"], (inp["gcn_b2"] @ inp["gcn_w3"])[None]], axis=0)
    fc3a = np.concatenate([inp["fc3_w"], inp["fc3_b"][None, :]], axis=0)

    wpack = np.zeros((128, WPACK_W), f32)
    wpack[0:128, W_IDB:W_IDB + 128] = np.eye(128)
    wpack[0:128, W_WZ:W_WZ + 64] = inp["w_z"]
    wpack[0:128, W_WR:W_WR + 64] = inp["w_r"]
    wpack[0:128, W_WH:W_WH + 64] = inp["w_h"]
    wpack[0:66, W_QA:W_QA + 256] = qA
    wpack[0:66, W_KA:W_KA + 256] = kA
    wpack[0:64, W_FC1:W_FC1 + 16] = inp["fc1_w"]
    wpack[0:16, W_FC2:W_FC2 + 2] = inp["fc2_w"]
    wpack[0:3, W_FC3A:W_FC3A + 64] = fc3a
    wpack[0:65, W_W1A:W_W1A + 64] = w1a
    wpack[0:65, W_W2A:W_W2A + 64] = w2a
    wpack[0:65, W_W3A:W_W3A + 64] = w3a
    wpack[0:64, W_SEL:W_SEL + 1] = 1.0
    wpack[64:128, W_SEL + 1:W_SEL + 2] = 1.0
    wpack[:, W_ONE:W_ONE + 128] = 1.0

    fpack = np.zeros((128, FPACK_W), f32)
    fpack[0:128, F_IDF:F_IDF + 128] = np.eye(128)
    fpack[0:16, F_B + 0] = inp["fc1_b"]
    fpack[0:2, F_B + 1] = inp["fc2_b"]
    fpack[0:64, F_B + 2] = inp["fc3_b"]
    fpack[0:128, F_EPS] = EPS
    fpack[0:64, F_XG] = inp["x_nom_g"]
    fpack[0:64, F_XB3] = inp["x_nom_b"] + inp["gcn_b3"]
    for k, nm in enumerate(("bn_g", "bn_b", "last_nom_g", "last_nom_b")):
        fpack[0, F_BN + 64 * k:F_BN + 64 * (k + 1)] = inp[nm]

    def c(a, dt=bf):
        return np.ascontiguousarray(np.asarray(a, dt))

    shared = {"wpack": c(wpack), "fpack": c(fpack, f32)}
    in_maps = []
    for core in range(8):
        bi, h = core // 2, core % 2
        off = h * HALF
        corr4 = np.stack([
            np.roll(inp["attn_norm_g"], -off),
            np.roll(inp["skip_norm_g"], -off),
            np.roll(inp["attn_norm_b"] + inp["skip_norm_b"], -off),
            np.ones(N, f32),
        ])
        m = dict(shared)
        m["xT"] = c(np.roll(inp["x"][bi], -off, axis=0).T)
        m["lastT"] = c(np.roll(inp["last_G_emb"][bi], -off, axis=0).T)
        m["origT"] = c(inp["orig_x"][bi, off:off + HALF].T)
        m["corr4"] = c(corr4)
        in_maps.append(m)
    return in_maps


def run(inputs, trace=False):
    nc = _build()
    in_maps = _host_prep(inputs)
    res = run_bass_kernel_spmd(nc, in_maps, core_ids=list(range(8)), trace=trace)
    out = np.zeros((B, N, E), np.float32)
    last = np.zeros((B, N, G), np.float32)
    for core in range(8):
        bi, h = core // 2, core % 2
        off = h * HALF
        out[bi, off:off + HALF] = res.results[core]["outH"]
        last[bi, off:off + HALF] = res.results[core]["lastH"]
    return (out, last), res


def kernel(**inputs):
    return run(inputs)[0]


# revision 18
# speedup vs baseline: 1.1823x; 1.1035x over previous
"""Trainium2 Bass kernel for nn_DGCN (gnn_message_passing).

Sharding: 8 shards = (batch b in 0..3, row-half h in 0..1). Each core gets
the full 2048-node K-side tensors of its batch with the node axis ROTATED
by h*1024 so the adjacency diagonal lands at the same tile position on
every core (uniform SPMD program); the core computes rows 0..1023 of the
rotated order, which are rows [h*1024, (h+1)*1024) of the original order.

v4 — breadth-first emission (engine queues are strict FIFO; depth-first
emission head-of-line blocks every queue), row-layout LN statistics:
 - All heavy tensors bf16; host pre-transposes x/last/orig; small params
   arrive in two packed mega-tiles (one bf16, one fp32) = 2 DMAs.
 - Head-mix scalars c_h and the Hg-LayerNorm gain fold into q_w/k_w; the
   LN shift becomes an extra contraction row of an augmented [66 x 256]
   weight (rhs rows = [Hg*a ; c ; 1]).
 - Per-node LN stats (Hg, xo, att/soc rows) are computed as [1/2, 512]
   PSUM rows by matmuls against ones/selector weights, scattered into
   [8/16, 128] chunk layout by SBUF-SBUF DMA for the pointwise math, and
   scattered back as broadcast rows fed to K=1 matmuls.
 - xo LayerNorm runs in T layout (feature axis on partitions) with the
   gain applied as a per-partition activation scale; no row-major xo.
 - diag(L)_i = relu(bracket)_ii / rowsum_j relu(bracket)_ij is invariant
   to positive per-row scales, so the 1/sdA row scale of the fused
   pre-relu matrix cancels; only the x3 lhsT rows carry rsS/rsA and the
   stat rows carry {-muA, -rho*muS, sdA}.
 - relu row-sums via accum_out; GCN biases ride the next layer's matmul
   as host-folded b@W rows against a constant ones row.
 - Phase I is software-pipelined two chunks deep over all 8 PSUM banks.
"""

import sys

if '/opt/trn_rl_repo' not in sys.path:
    sys.path.insert(0, '/opt/trn_rl_repo')

from contextlib import ExitStack

import numpy as np
import ml_dtypes

import concourse.bass as bass
import concourse.tile as tile
from concourse import bacc, mybir
from concourse.bass_interp import get_hw_module
from concourse.bass_utils import run_bass_kernel_spmd

FP = mybir.dt.float32
BF = mybir.dt.bfloat16
AL = mybir.AluOpType
AF = mybir.ActivationFunctionType
AX = mybir.AxisListType

B, N, E, G, H = 4, 2048, 64, 64, 4
D = H * G          # 256
HALF = N // 2      # own rows per core
NCH = N // 128     # 16 chunks over all nodes
HCH = HALF // 128  # 8 own chunks
MB = 512
NMB = N // MB      # 4
EPS = 1e-5

# wpack (bf16 [128, WPACK_W]) column layout
W_IDB, W_WZ, W_WR, W_WH = 0, 128, 192, 256
W_QA, W_KA = 320, 576
W_FC1, W_FC2, W_FC3A = 832, 848, 850
W_W1A, W_W2A, W_W3A = 914, 978, 1042
W_SEL, W_ONE = 1106, 1108
WPACK_W = 1280
# fpack (fp32 [128, FPACK_W]) column layout
F_IDF, F_B, F_EPS, F_XG, F_XB3, F_BN = 0, 128, 132, 133, 134, 136
FPACK_W = 528

_CACHE = {}


def _tp(nc, out_ap, in_ap, ident):
    k = in_ap.partition_size()
    nc.tensor.transpose(out_ap, in_ap, ident[0:k, 0:k])


def _leaky(nc, dst):
    nc.vector.scalar_tensor_tensor(dst, dst, 0.01, dst, AL.mult, AL.max)


def _ln_rows(nc, pool, t_in, t_out, g_b, b_b, ngr, tag, epsc):
    """LayerNorm over 64-wide groups: t_in [128, ngr*64] -> t_out."""
    a3 = t_in[:].rearrange("p (g e) -> p g e", e=64)
    o3 = t_out[:].rearrange("p (g e) -> p g e", e=64)
    sm = pool.tile([128, ngr], FP, tag=f"{tag}_sm")
    nc.vector.tensor_reduce(sm[:], a3, AX.X, AL.add)
    sq = pool.tile([128, ngr * 64], FP, tag=f"{tag}_sq")
    nc.scalar.square(sq[:], t_in[:])
    sqs = pool.tile([128, ngr], FP, tag=f"{tag}_sqs")
    nc.vector.tensor_reduce(sqs[:], sq[:].rearrange("p (g e) -> p g e", e=64),
                            AX.X, AL.add)
    mu = pool.tile([128, ngr], FP, tag=f"{tag}_mu")
    nc.vector.tensor_scalar(mu[:], sm[:], 1.0 / 64, None, AL.mult)
    mu2 = pool.tile([128, ngr], FP, tag=f"{tag}_mu2")
    nc.vector.tensor_tensor(mu2[:], mu[:], mu[:], AL.mult)
    var = pool.tile([128, ngr], FP, tag=f"{tag}_var")
    nc.vector.scalar_tensor_tensor(var[:], sqs[:], 1.0 / 64, mu2[:], AL.mult,
                                   AL.subtract)
    sd = pool.tile([128, ngr], FP, tag=f"{tag}_sd")
    nc.scalar.activation(sd[:], var[:], AF.Sqrt, bias=epsc)
    rs = pool.tile([128, ngr], FP, tag=f"{tag}_rs")
    nc.vector.reciprocal(rs[:], sd[:])
    mu_b = mu[:].unsqueeze(2).broadcast_to([128, ngr, 64])
    rs_b = rs[:].unsqueeze(2).broadcast_to([128, ngr, 64])
    g3 = g_b.unsqueeze(1).broadcast_to([128, ngr, 64])
    b3 = b_b.unsqueeze(1).broadcast_to([128, ngr, 64])
    xc = pool.tile([128, ngr * 64], FP, tag=f"{tag}_xc")
    xc3 = xc[:].rearrange("p (g e) -> p g e", e=64)
    nc.vector.tensor_tensor(xc3, a3, mu_b, AL.subtract)
    nc.vector.tensor_tensor(xc3, xc3, rs_b, AL.mult)
    nc.vector.tensor_tensor(xc3, xc3, g3, AL.mult)
    nc.vector.tensor_tensor(o3, xc3, b3, AL.add)


def _stat_land(nc, small, sum_t, sq_t, nch, tag, epsc, inv):
    """[nch,128] sums/sumsq -> (a, c) = (1/sd, -mu/sd), both [nch, 128] fp32."""
    mu = small.tile([nch, 128], FP, tag=f"{tag}_mu", name=f"{tag}_mu")
    nc.vector.tensor_scalar(mu[:], sum_t, inv, None, AL.mult)
    mu2 = small.tile([nch, 128], FP, tag=f"{tag}_mu2", name=f"{tag}_mu2")
    nc.vector.tensor_tensor(mu2[:], mu[:], mu[:], AL.mult)
    var = small.tile([nch, 128], FP, tag=f"{tag}_var", name=f"{tag}_var")
    nc.vector.scalar_tensor_tensor(var[:], sq_t, inv, mu2[:], AL.mult, AL.subtract)
    sd = small.tile([nch, 128], FP, tag=f"{tag}_sd", name=f"{tag}_sd")
    nc.scalar.activation(sd[:], var[:], AF.Sqrt, bias=epsc)
    a = small.tile([nch, 128], FP, tag=f"{tag}_a", name=f"{tag}_a")
    nc.vector.reciprocal(a[:], sd[:])
    c = small.tile([nch, 128], FP, tag=f"{tag}_c", name=f"{tag}_c")
    nc.vector.scalar_tensor_tensor(c[:], mu[:], -1.0, a[:], AL.mult, AL.mult)
    return mu, sd, a, c


def _emit(ctx: ExitStack, tc: tile.TileContext, io: dict):
    nc = tc.nc

    persist = ctx.enter_context(tc.tile_pool(name="persist", bufs=1))
    small = ctx.enter_context(tc.tile_pool(name="small", bufs=1))

    # ---------------- packed params (2 DMAs) ----------------
    wp = persist.tile([128, WPACK_W], BF, tag="wp")
    nc.sync.dma_start(wp[:], io["wpack"][:])
    fp_ = persist.tile([128, FPACK_W], FP, tag="fp_")
    nc.scalar.dma_start(fp_[:], io["fpack"][:])

    identb = wp[:, W_IDB:W_IDB + 128]
    wz = wp[:, W_WZ:W_WZ + 64]
    wr = wp[:, W_WR:W_WR + 64]
    wh = wp[:, W_WH:W_WH + 64]
    kA = wp[0:66, W_KA:W_KA + 256]
    fc1s = wp[0:64, W_FC1:W_FC1 + 16]
    fc2s = wp[0:16, W_FC2:W_FC2 + 2]
    fc3s = wp[0:2, W_FC3A:W_FC3A + 64]
    fc3a = wp[0:3, W_FC3A:W_FC3A + 64]
    w1a = wp[0:65, W_W1A:W_W1A + 64]
    w2a = wp[0:65, W_W2A:W_W2A + 64]
    w3a = wp[0:65, W_W3A:W_W3A + 64]
    sel2 = wp[:, W_SEL:W_SEL + 2]
    ones128c = wp[:, W_ONE:W_ONE + 1]
    ones64c = wp[0:64, W_ONE:W_ONE + 1]
    onesr128 = wp[0:1, W_ONE:W_ONE + 128]
    onesr64 = wp[0:1, W_ONE:W_ONE + 64]

    ident = fp_[:, F_IDF:F_IDF + 128]
    fc1b = fp_[0:16, F_B + 0:F_B + 1]
    fc2b = fp_[0:2, F_B + 1:F_B + 2]
    fc3b = fp_[0:64, F_B + 2:F_B + 3]
    epsc128 = fp_[0:128, F_EPS:F_EPS + 1]
    epsc16 = fp_[0:16, F_EPS:F_EPS + 1]
    epsc8 = fp_[0:8, F_EPS:F_EPS + 1]
    xng_c = fp_[0:64, F_XG:F_XG + 1]
    xb3_c = fp_[0:64, F_XB3:F_XB3 + 1]

    # LN parameter rows -> [128, 64] broadcast tiles via gpsimd (small)
    brows = {}
    for k, nm in enumerate(("bng", "bnb", "lng", "lnb")):
        t = persist.tile([128, 64], FP, tag=f"{nm}_b", name=f"{nm}_b")
        nc.gpsimd.partition_broadcast(
            t[:], fp_[0:1, F_BN + 64 * k:F_BN + 64 * (k + 1)])
        brows[nm] = t

    # ---------------- big persistent tiles ----------------
    xT = persist.tile([64, N], BF, tag="xT")
    lastT = persist.tile([64, N], BF, tag="lastT")
    c1 = persist.tile([128, N], BF, tag="c1")      # [x3 ; last]
    c2 = persist.tile([128, N], BF, tag="c2")      # [r*last ; x3]
    hgsq = persist.tile([128, N], BF, tag="hgsq")  # [Hg_raw ; Hg_raw^2]
    HgQ = persist.tile([66, N], BF, tag="HgQ")     # [Hg*a ; c ; 1]
    osq = persist.tile([128, HALF], BF, tag="osq")  # [origT ; origT^2]
    a_row = persist.tile([1, N], BF, tag="a_row")
    kT0 = persist.tile([128, N], BF, tag="kT0")
    kT1 = persist.tile([128, N], BF, tag="kT1")
    k2T0 = persist.tile([128, N], BF, tag="k2T0")
    k2T1 = persist.tile([128, N], BF, tag="k2T1")
    qT0 = persist.tile([128, HALF], BF, tag="qT0")
    qT1 = persist.tile([128, HALF], BF, tag="qT1")
    x3gs = persist.tile([67, N], BF, tag="x3gs")   # [x3*gs ; ga ; gs ; cb]
    x3rA = persist.tile([67, HALF], BF, tag="x3rA")
    ga_b = persist.tile([128, N], BF, tag="ga_b")
    gs_b = persist.tile([64, N], BF, tag="gs_b")
    ga_r = persist.tile([1, N], BF, tag="ga_r")
    gs_r = persist.tile([1, N], BF, tag="gs_r")
    gt_sb = persist.tile([128, 256], BF, tag="gt_sb")
    gb_sb = persist.tile([128, 256], BF, tag="gb_sb")
    gs_f = persist.tile([64, 64], BF, tag="gs_f")
    ks0 = persist.tile([128, 1], BF, tag="ks0")
    ks1 = persist.tile([128, 1], BF, tag="ks1")
    xsb = persist.tile([64, 1], BF, tag="xsb")
    rc32 = persist.tile([128, 4 * HCH], FP, tag="rc32")
    dg8 = persist.tile([128, HCH], FP, tag="dg8")
    dl = persist.tile([128, HCH], FP, tag="dl")
    x1T = persist.tile([16, N], BF, tag="x1T")
    x2a = persist.tile([3, N], BF, tag="x2a")      # [x2 ; 1]
    e0sb = persist.tile([128, HALF], BF, tag="e0sb")
    e1sb = persist.tile([128, HALF], BF, tag="e1sb")
    essb = persist.tile([64, HALF], BF, tag="essb")
    ph1sb = persist.tile([64, HALF], BF, tag="ph1sb")
    finsq = persist.tile([128, HALF], BF, tag="finsq")
    cT_sb = persist.tile([128, NCH], FP, tag="cT_sb")
    lastR = persist.tile([128, HCH * 64], FP, tag="lastR")
    x1aug = persist.tile([65, HALF], BF, tag="x1aug")  # [xo^T + b3 ; 1]
    hca = persist.tile([65, HALF], BF, tag="hca")
    hcb = persist.tile([65, HALF], BF, tag="hcb")
    fin = persist.tile([128, HCH * 64], FP, tag="fin")

    # input loads
    nc.sync.dma_start(xT[:], io["xT"][:])
    nc.sync.dma_start(lastT[:], io["lastT"][:])
    nc.sync.dma_start(c1[64:128, :], io["lastT"][:])
    nc.sync.dma_start(osq[0:64, :], io["origT"][:])
    nc.sync.dma_start(x3gs[64:67, :], io["corr4"][0:3, :])
    nc.sync.dma_start(ga_r[:], io["corr4"][0:1, :])
    nc.sync.dma_start(gs_r[:], io["corr4"][1:2, :])
    # constant-ones rows
    nc.gpsimd.dma_start(HgQ[65:66, :], io["corr4"][3:4, :])
    nc.gpsimd.dma_start(x2a[2:3, :], io["corr4"][3:4, :])
    nc.gpsimd.dma_start(x1aug[64:65, :], io["corr4"][3:4, 0:HALF])
    nc.gpsimd.dma_start(hca[64:65, :], io["corr4"][3:4, 0:HALF])
    nc.gpsimd.dma_start(hcb[64:65, :], io["corr4"][3:4, 0:HALF])

    frontA = ExitStack()
    fps = frontA.enter_context(tc.tile_pool(name="fps", bufs=6, space="PSUM"))
    gw = frontA.enter_context(tc.tile_pool(name="gw", bufs=4))

    MBs = [slice(j * MB, (j + 1) * MB) for j in range(NMB)]
    HBs = [slice(j * MB, (j + 1) * MB) for j in range(2)]

    # ---- ga / gs broadcast tiles via K=1 matmuls (breadth) ----
    gps_ = [fps.tile([128, MB], FP, tag="fp", name=f"gab_{j}") for j in range(NMB)]
    gss_ = [fps.tile([64, MB], FP, tag="fp", name=f"gsb_{j}") for j in range(2)]
    for j in range(NMB):
        nc.tensor.matmul(gps_[j][:], onesr128, ga_r[:, MBs[j]], start=True, stop=True)
    for j in range(2):
        nc.tensor.matmul(gss_[j][:], onesr64, gs_r[:, j * MB:(j + 1) * MB],
                         start=True, stop=True)
    for j in range(NMB):
        (nc.vector.tensor_copy if j % 2 == 0 else nc.scalar.copy)(
            ga_b[:, MBs[j]], gps_[j][:])
    for j in range(2):
        (nc.scalar.copy if j % 2 == 0 else nc.vector.tensor_copy)(
            gs_b[:, slice(j * MB, (j + 1) * MB)], gss_[j][:])
    gss2_ = [fps.tile([64, MB], FP, tag="fp", name=f"gsb2_{j}") for j in range(2)]
    for j in range(2):
        sl = slice((2 + j) * MB, (3 + j) * MB)
        nc.tensor.matmul(gss2_[j][:], onesr64, gs_r[:, sl], start=True, stop=True)
        (nc.vector.tensor_copy if j % 2 == 0 else nc.scalar.copy)(
            gs_b[:, sl], gss2_[j][:])

    # ---- xo stats (input-only dependent, fills the early pipeline) ----
    nc.scalar.square(osq[64:128, :], osq[0:64, :])
    oxp = [fps.tile([2, MB], FP, tag="fp", name=f"oxp_{j}") for j in range(2)]
    for j in range(2):
        nc.tensor.matmul(oxp[j][:], sel2, osq[:, HBs[j]], start=True, stop=True)
    oxs = small.tile([2, HALF], FP, tag="oxs")
    for j in range(2):
        (nc.vector.tensor_copy if j == 0 else nc.scalar.copy)(oxs[:, HBs[j]], oxp[j][:])
    oxs0 = small.tile([HCH, 128], FP, tag="oxs0")
    nc.sync.dma_start(oxs0[:], oxs[0:1, :].rearrange("o (i p) -> o i p", p=128))
    oxs1 = small.tile([HCH, 128], FP, tag="oxs1")
    nc.sync.dma_start(oxs1[:], oxs[1:2, :].rearrange("o (i p) -> o i p", p=128))
    _, _, oa, oc = _stat_land(nc, small, oxs0[:], oxs1[:], HCH, "ox", epsc8, 1.0 / 64)
    oa8 = small.tile([HCH, 128], BF, tag="oa8")
    nc.vector.tensor_copy(oa8[:], oa[:])
    oc8 = small.tile([HCH, 128], BF, tag="oc8")
    nc.scalar.copy(oc8[:], oc[:])
    oar = small.tile([1, HALF], BF, tag="oar")
    nc.sync.dma_start(oar[:].rearrange("o (i p) -> o i p", p=128), oa8[:])
    ocr = small.tile([1, HALF], BF, tag="ocr")
    nc.sync.dma_start(ocr[:].rearrange("o (i p) -> o i p", p=128), oc8[:])

    # ============ hyper fc stack (breadth-first stages) ============
    xacc = small.tile([64, NMB], FP, tag="xacc")
    p1 = [fps.tile([16, MB], FP, tag="fp", name=f"p1_{j}") for j in range(NMB)]
    for j in range(NMB):
        nc.tensor.matmul(p1[j][:], fc1s, xT[:, MBs[j]], start=True, stop=True)
    for j in range(NMB):
        nc.scalar.activation(x1T[:, MBs[j]], p1[j][:], AF.Sigmoid, bias=fc1b)
    p2 = [fps.tile([2, MB], FP, tag="fp", name=f"p2_{j}") for j in range(NMB)]
    for j in range(NMB):
        nc.tensor.matmul(p2[j][:], fc2s, x1T[:, MBs[j]], start=True, stop=True)
    for j in range(NMB):
        nc.scalar.activation(x2a[0:2, MBs[j]], p2[j][:], AF.Sigmoid, bias=fc2b)
    p3 = [fps.tile([64, MB], FP, tag="fp", name=f"p3_{j}") for j in range(NMB)]
    for j in range(NMB):
        nc.tensor.matmul(p3[j][:], fc3s, x2a[0:2, MBs[j]], start=True, stop=True)
    for j in range(NMB):
        nc.scalar.activation(c1[0:64, MBs[j]], p3[j][:], AF.Identity, bias=fc3b,
                             accum_out=xacc[:, j:j + 1])
    for j in range(NMB):
        nc.vector.tensor_copy(c2[64:128, MBs[j]], c1[0:64, MBs[j]])
    xs_f = small.tile([64, 1], FP, tag="xs_f")
    nc.vector.tensor_reduce(xs_f[:], xacc[:], AX.X, AL.add)
    nc.vector.tensor_copy(xsb[:], xs_f[:])

    # ================= GRU gates (breadth-first stages) =================
    zp = [fps.tile([64, MB], FP, tag="fp", name=f"zp_{j}") for j in range(NMB)]
    for j in range(NMB):
        nc.tensor.matmul(zp[j][:], wz, c1[:, MBs[j]], start=True, stop=True)
    zt = [gw.tile([64, MB], BF, tag="zt", name=f"zt_{j}") for j in range(NMB)]
    for j in range(NMB):
        nc.scalar.activation(zt[j][:], zp[j][:], AF.Sigmoid)
    rp = [fps.tile([64, MB], FP, tag="fp", name=f"rp_{j}") for j in range(NMB)]
    for j in range(NMB):
        nc.tensor.matmul(rp[j][:], wr, c1[:, MBs[j]], start=True, stop=True)
    rt = [gw.tile([64, MB], BF, tag="rt", name=f"rt_{j}") for j in range(NMB)]
    for j in range(NMB):
        nc.scalar.activation(rt[j][:], rp[j][:], AF.Sigmoid)
    for j in range(NMB):
        nc.vector.tensor_tensor(c2[0:64, MBs[j]], rt[j][:], lastT[:, MBs[j]], AL.mult)
    hp = [fps.tile([64, MB], FP, tag="fp", name=f"hp_{j}") for j in range(NMB)]
    for j in range(NMB):
        nc.tensor.matmul(hp[j][:], wh, c2[:, MBs[j]], start=True, stop=True)
    ht = [gw.tile([64, MB], BF, tag="ht", name=f"ht_{j}") for j in range(NMB)]
    for j in range(NMB):
        nc.scalar.activation(ht[j][:], hp[j][:], AF.Tanh)
    dt_ = [gw.tile([64, MB], BF, tag="dt", name=f"dt_{j}") for j in range(NMB)]
    for j in range(NMB):
        nc.vector.tensor_tensor(dt_[j][:], ht[j][:], lastT[:, MBs[j]], AL.subtract)
    for j in range(NMB):
        nc.vector.tensor_tensor(dt_[j][:], dt_[j][:], zt[j][:], AL.mult)
    for j in range(NMB):
        nc.vector.tensor_tensor(hgsq[0:64, MBs[j]], dt_[j][:], lastT[:, MBs[j]], AL.add)

    # Hg^2 (scalar) then Hg LN stats rows
    nc.scalar.square(hgsq[64:128, 0:HALF], hgsq[0:64, 0:HALF])
    nc.vector.tensor_tensor(hgsq[64:128, HALF:N], hgsq[0:64, HALF:N],
                            hgsq[0:64, HALF:N], AL.mult)
    hsp = [fps.tile([2, MB], FP, tag="fp", name=f"hsp_{j}") for j in range(NMB)]
    for j in range(NMB):
        nc.tensor.matmul(hsp[j][:], sel2, hgsq[:, MBs[j]], start=True, stop=True)
    hsum = small.tile([2, N], FP, tag="hsum")
    for j in range(NMB):
        (nc.vector.tensor_copy if j % 2 == 0 else nc.scalar.copy)(
            hsum[:, MBs[j]], hsp[j][:])
    hst0 = small.tile([NCH, 128], FP, tag="hst0")
    nc.sync.dma_start(hst0[:], hsum[0:1, :].rearrange("o (i p) -> o i p", p=128))
    hst1 = small.tile([NCH, 128], FP, tag="hst1")
    nc.sync.dma_start(hst1[:], hsum[1:2, :].rearrange("o (i p) -> o i p", p=128))
    _, _, ha, hc = _stat_land(nc, small, hst0[:], hst1[:], NCH, "hg", epsc16, 1.0 / 64)
    ha16 = small.tile([NCH, 128], BF, tag="ha16")
    nc.vector.tensor_copy(ha16[:], ha[:])
    hc16 = small.tile([NCH, 128], BF, tag="hc16")
    nc.scalar.copy(hc16[:], hc[:])
    nc.sync.dma_start(a_row[:].rearrange("o (i p) -> o i p", p=128), ha16[:])
    nc.sync.dma_start(HgQ[64:65, :].rearrange("o (i p) -> o i p", p=128), hc16[:])
    # c in chunk-column layout for the lastH bias path
    pcc = fps.tile([128, NCH], FP, tag="fp", name="pcc", padded_shape=[128, 512])
    _tp(nc, pcc[:], hc[:], ident)
    nc.scalar.copy(cT_sb[:], pcc[:])

    # HgA = Hg_raw * a (K=1 broadcast matmul + fused multiply from PSUM)
    ab = [fps.tile([64, MB], FP, tag="fp", name=f"ab_{j}") for j in range(NMB)]
    for j in range(NMB):
        nc.tensor.matmul(ab[j][:], onesr64, a_row[:, MBs[j]], start=True, stop=True)
    for j in range(NMB):
        nc.vector.tensor_tensor(HgQ[0:64, MBs[j]], hgsq[0:64, MBs[j]], ab[j][:],
                                AL.mult)

    # ===================== q / k projections =====================
    kacc = small.tile([128, 8], FP, tag="kacc")
    kjobs = []
    for half, dst in ((0, kT0), (1, kT1)):
        for j in range(NMB):
            kjobs.append((dst, slice(W_KA + 128 * half, W_KA + 128 * (half + 1)),
                          MBs[j], kacc[:, 4 * half + j:4 * half + j + 1]))
    qjobs = []
    for half, dst in ((0, qT0), (1, qT1)):
        for j in range(2):
            qjobs.append((dst, slice(W_QA + 128 * half, W_QA + 128 * (half + 1)),
                          HBs[j], None))
    kq_ps = []
    for idx, (dst, wsl, sl, acc) in enumerate(kjobs + qjobs):
        kp = fps.tile([128, MB], FP, tag="fp", name=f"kqp_{idx}")
        nc.tensor.matmul(kp[:], wp[0:66, wsl], HgQ[:, sl], start=True, stop=True)
        kq_ps.append(kp)
    for idx, (dst, wsl, sl, acc) in enumerate(kjobs + qjobs):
        nc.scalar.copy(dst[:, sl], kq_ps[idx][:])
        if acc is not None:
            nc.vector.scalar_tensor_tensor(dst[:, sl], dst[:, sl], 0.01, dst[:, sl],
                                           AL.mult, AL.max, accum_out=acc)
        else:
            _leaky(nc, dst[:, sl])
    ks_f = small.tile([128, 2], FP, tag="ks_f")
    nc.vector.tensor_reduce(ks_f[:], kacc[:].rearrange("p (h j) -> p h j", j=4),
                            AX.X, AL.add)
    nc.vector.tensor_copy(ks0[:], ks_f[:, 0:1])
    nc.vector.tensor_copy(ks1[:], ks_f[:, 1:2])
    # k2 = k * ga ; x3gs rows 0:64 = x3 * gs
    nc.vector.tensor_tensor(k2T0[:], kT0[:], ga_b[:], AL.mult)
    nc.vector.tensor_tensor(k2T1[:], kT1[:], ga_b[:], AL.mult)
    nc.vector.tensor_tensor(x3gs[0:64, :], c1[0:64, :], gs_b[:], AL.mult)

    frontA.close()

    # ===================== Gram matrices =====================
    with tc.tile_pool(name="gpsp", bufs=3, space="PSUM") as gpsp, \
         tc.tile_pool(name="krpp", bufs=3, space="PSUM") as krpp, \
         tc.tile_pool(name="krp", bufs=3) as krp:
        gt_ps = gpsp.tile([128, 256], FP, tag="g", padded_shape=[128, 512])
        gb_ps = gpsp.tile([128, 256], FP, tag="g", padded_shape=[128, 512])
        for mi in range(NCH):
            msl = slice(mi * 128, (mi + 1) * 128)
            krq = krpp.tile([128, 256], FP, tag="kr", padded_shape=[128, 512])
            nc.tensor.matmul(krq[:], HgQ[:, msl], kA, start=True, stop=True)
            kr = krp.tile([128, 256], BF, tag="kr")
            nc.scalar.copy(kr[:], krq[:])
            _leaky(nc, kr[:])
            nc.tensor.matmul(gt_ps[:], kr[:, 0:128], kr[:],
                             start=(mi == 0), stop=(mi == NCH - 1))
            nc.tensor.matmul(gb_ps[:], kr[:, 128:256], kr[:],
                             start=(mi == 0), stop=(mi == NCH - 1))
        nc.vector.tensor_copy(gt_sb[:], gt_ps[:])
        nc.scalar.copy(gb_sb[:], gb_ps[:])
        gs_ps = gpsp.tile([64, 64], FP, tag="g", padded_shape=[64, 512])
        for mi in range(NCH):
            msl = slice(mi * 128, (mi + 1) * 128)
            xrq = krpp.tile([128, 64], FP, tag="kr", padded_shape=[128, 512])
            nc.tensor.matmul(xrq[:], x2a[:, msl], fc3a, start=True, stop=True)
            xr = krp.tile([128, 64], BF, tag="xr")
            (nc.vector.tensor_copy if mi % 2 == 0 else nc.scalar.copy)(xr[:], xrq[:])
            nc.tensor.matmul(gs_ps[:], xr[:], xr[:],
                             start=(mi == 0), stop=(mi == NCH - 1))
        nc.vector.tensor_copy(gs_f[:], gs_ps[:])

    # ============== own-row stats: S1, T1, S2, T2 rows ==============
    statq = ExitStack()
    ups = statq.enter_context(tc.tile_pool(name="ups", bufs=2, space="PSUM"))
    sps = statq.enter_context(tc.tile_pool(name="sps", bufs=2, space="PSUM"))
    lps = statq.enter_context(tc.tile_pool(name="lps", bufs=2, space="PSUM"))
    # u = G q per 512-half; e = u * q  (separate e0/e1, summed in PSUM below)
    for jb in range(2):
        sl = HBs[jb]
        ut0 = ups.tile([128, MB], FP, tag="ut", name=f"ut0_{jb}")
        nc.tensor.matmul(ut0[:], gt_sb[:, 0:128], qT0[:, sl], start=True, stop=False)
        nc.tensor.matmul(ut0[:], gb_sb[:, 0:128], qT1[:, sl], start=False, stop=True)
        ut1 = ups.tile([128, MB], FP, tag="ut", name=f"ut1_{jb}")
        nc.tensor.matmul(ut1[:], gt_sb[:, 128:256], qT0[:, sl], start=True, stop=False)
        nc.tensor.matmul(ut1[:], gb_sb[:, 128:256], qT1[:, sl], start=False, stop=True)
        nc.vector.tensor_tensor(e0sb[:, sl], ut0[:], qT0[:, sl], AL.mult)
        nc.vector.tensor_tensor(e1sb[:, sl], ut1[:], qT1[:, sl], AL.mult)
    for jb in range(2):
        sl = HBs[jb]
        us = ups.tile([64, MB], FP, tag="ut", name=f"us_{jb}")
        nc.tensor.matmul(us[:], gs_f[:], c1[0:64, sl], start=True, stop=True)
        nc.vector.tensor_tensor(essb[:, sl], us[:], c1[0:64, sl], AL.mult)

    s1sb = small.tile([1, HALF], FP, tag="s1sb")
    t1sb = small.tile([1, HALF], FP, tag="t1sb")
    s2sb = small.tile([1, HALF], FP, tag="s2sb")
    t2sb = small.tile([1, HALF], FP, tag="t2sb")
    for jb in range(2):
        sl = HBs[jb]
        s1p = sps.tile([1, MB], FP, tag="st", name=f"s1p_{jb}", padded_shape=[1, 512])
        nc.tensor.matmul(s1p[:], ks0[:], qT0[:, sl], start=True, stop=False)
        nc.tensor.matmul(s1p[:], ks1[:], qT1[:, sl], start=False, stop=True)
        t1p = sps.tile([1, MB], FP, tag="st", name=f"t1p_{jb}", padded_shape=[1, 512])
        nc.tensor.matmul(t1p[:], xsb[:], c1[0:64, sl], start=True, stop=True)
        nc.scalar.copy(s1sb[:, sl], s1p[:])
        nc.vector.tensor_copy(t1sb[:, sl], t1p[:])
    for jb in range(2):
        sl = HBs[jb]
        s2p = sps.tile([1, MB], FP, tag="st", name=f"s2p_{jb}", padded_shape=[1, 512])
        nc.tensor.matmul(s2p[:], ones128c, e0sb[:, sl], start=True, stop=False)
        nc.tensor.matmul(s2p[:], ones128c, e1sb[:, sl], start=False, stop=True)
        t2p = sps.tile([1, MB], FP, tag="st", name=f"t2p_{jb}", padded_shape=[1, 512])
        nc.tensor.matmul(t2p[:], ones64c, essb[:, sl], start=True, stop=True)
        nc.scalar.copy(s2sb[:, sl], s2p[:])
        nc.vector.tensor_copy(t2sb[:, sl], t2p[:])
    s1t = small.tile([HCH, 128], FP, tag="s1t")
    nc.sync.dma_start(s1t[:], s1sb[:].rearrange("o (i p) -> o i p", p=128))
    t1t = small.tile([HCH, 128], FP, tag="t1t")
    nc.scalar.dma_start(t1t[:], t1sb[:].rearrange("o (i p) -> o i p", p=128))
    s2t = small.tile([HCH, 128], FP, tag="s2t")
    nc.sync.dma_start(s2t[:], s2sb[:].rearrange("o (i p) -> o i p", p=128))
    t2t = small.tile([HCH, 128], FP, tag="t2t")
    nc.scalar.dma_start(t2t[:], t2sb[:].rearrange("o (i p) -> o i p", p=128))

    # ---- fillers for the stats-land latency ----
    # lastH output (Hg LN rows, own half)
    for i in range(HCH):
        pt = lps.tile([128, 64], BF, tag="lpt", name=f"lpt_{i}",
                      padded_shape=[128, 1024])
        _tp(nc, pt[:], HgQ[0:64, i * 128:(i + 1) * 128], identb)
        nc.scalar.activation(lastR[:, i * 64:(i + 1) * 64], pt[:], AF.Identity,
                             bias=cT_sb[:, i:i + 1])
    l3 = lastR[:].rearrange("p (g e) -> p g e", e=64)
    lg3 = brows["bng"][:].unsqueeze(1).broadcast_to([128, HCH, 64])
    lb3 = brows["bnb"][:].unsqueeze(1).broadcast_to([128, HCH, 64])
    nc.vector.tensor_tensor(l3, l3, lg3, AL.mult)
    nc.vector.tensor_tensor(l3, l3, lb3, AL.add)
    nc.sync.dma_start(io["lastH"].rearrange("(i p) e -> p i e", p=128),
                      lastR[:].rearrange("p (i e) -> p i e", e=64))

    # xo affine into x1aug
    oab = [sps.tile([64, MB], FP, tag="st", name=f"oab_{j}") for j in range(2)]
    for j in range(2):
        nc.tensor.matmul(oab[j][:], onesr64, oar[:, HBs[j]], start=True, stop=True)
    ocb = [sps.tile([64, MB], FP, tag="st", name=f"ocb_{j}") for j in range(2)]
    for j in range(2):
        nc.tensor.matmul(ocb[j][:], onesr64, ocr[:, HBs[j]], start=True, stop=True)
    for j in range(2):
        tb = small.tile([64, MB], BF, tag=f"oxt_{j}", name=f"oxt_{j}")
        nc.vector.tensor_tensor(tb[:], osq[0:64, HBs[j]], oab[j][:], AL.mult)
        nc.vector.tensor_tensor(tb[:], tb[:], ocb[j][:], AL.add)
        nc.scalar.activation(x1aug[0:64, HBs[j]], tb[:], AF.Identity,
                             scale=xng_c, bias=xb3_c)

    # GCN layer-1 matmul (dl-independent)
    for jb in range(2):
        ph1 = sps.tile([64, MB], FP, tag="st", name=f"ph1_{jb}")
        nc.tensor.matmul(ph1[:], w1a, x1aug[:, HBs[jb]], start=True, stop=True)
        (nc.scalar.copy if jb == 0 else nc.vector.tensor_copy)(
            ph1sb[:, HBs[jb]], ph1[:])

    # ---- own stats land [8, 128] ----
    muA, sdA, rsA_, _cA = _stat_land(nc, small, s1t[:], s2t[:], HCH, "sa",
                                     epsc8, 1.0 / N)
    muS, sdS, rsS_, _cS = _stat_land(nc, small, t1t[:], t2t[:], HCH, "ss",
                                     epsc8, 1.0 / N)
    rho = small.tile([HCH, 128], FP, tag="rho")
    nc.vector.tensor_tensor(rho[:], rsS_[:], sdA[:], AL.mult)
    r64t = small.tile([HCH, 128], BF, tag="r64t")
    nc.vector.tensor_scalar(r64t[:], muA[:], -1.0, None, AL.mult)
    r65t = small.tile([HCH, 128], BF, tag="r65t")
    nc.vector.scalar_tensor_tensor(r65t[:], muS[:], -1.0, rho[:], AL.mult, AL.mult)
    r66t = small.tile([HCH, 128], BF, tag="r66t")
    nc.scalar.copy(r66t[:], sdA[:])
    rho16 = small.tile([HCH, 128], BF, tag="rho16")
    nc.scalar.copy(rho16[:], rho[:])
    nc.sync.dma_start(x3rA[64:65, :].rearrange("o (i p) -> o i p", p=128), r64t[:])
    nc.scalar.dma_start(x3rA[65:66, :].rearrange("o (i p) -> o i p", p=128), r65t[:])
    nc.sync.dma_start(x3rA[66:67, :].rearrange("o (i p) -> o i p", p=128), r66t[:])
    rho_row = small.tile([1, HALF], BF, tag="rho_row")
    nc.scalar.dma_start(rho_row[:].rearrange("o (i p) -> o i p", p=128), rho16[:])
    for jb in range(2):
        rp_ = sps.tile([64, MB], FP, tag="st", name=f"rhob_{jb}")
        nc.tensor.matmul(rp_[:], onesr64, rho_row[:, HBs[jb]], start=True, stop=True)
        nc.vector.tensor_tensor(x3rA[0:64, HBs[jb]], c1[0:64, HBs[jb]], rp_[:],
                                AL.mult)
    statq.close()

    # ========== phase I + interleaved per-half tail ==========
    with tc.tile_pool(name="zps", bufs=8, space="PSUM") as zps, \
         tc.tile_pool(name="scrv", bufs=2) as scrv, \
         tc.tile_pool(name="scra", bufs=2) as scra:
        ztiles = {}

        def passes12(i):
            csl = slice(i * 128, (i + 1) * 128)
            zpt = [zps.tile([128, MB], FP, tag="zpt", name=f"zp_{i}_{m}")
                   for m in range(NMB)]
            ztiles[i] = zpt
            for mb in range(NMB):
                nc.tensor.matmul(zpt[mb][:], qT0[:, csl],
                                 k2T0[:, mb * MB:(mb + 1) * MB],
                                 start=True, stop=False)
            for mb in range(NMB):
                nc.tensor.matmul(zpt[mb][:], qT1[:, csl],
                                 k2T1[:, mb * MB:(mb + 1) * MB],
                                 start=False, stop=False)

        def pass3(i):
            csl = slice(i * 128, (i + 1) * 128)
            zpt = ztiles[i]
            for mb in range(NMB):
                nc.tensor.matmul(zpt[mb][:], x3rA[:, csl],
                                 x3gs[:, mb * MB:(mb + 1) * MB],
                                 start=False, stop=True)
            for mb in range(NMB):
                acc = rc32[:, 4 * i + mb:4 * i + mb + 1]
                if mb % 2 == 0:
                    scr = scrv.tile([128, MB], FP, tag="scr", name=f"scr_{i}_{mb}")
                    nc.vector.tensor_scalar(scr[:], zpt[mb][:], 0.0, None, AL.max,
                                            AL.add, accum_out=acc)
                else:
                    scr = scra.tile([128, MB], FP, tag="scr2", name=f"scr2_{i}_{mb}")
                    nc.scalar.activation(scr[:], zpt[mb][:], AF.Relu, accum_out=acc)
                if mb == i // 4:
                    off = (i * 128) % MB
                    dsel = scrv.tile([128, 128], FP, tag="dsel", name=f"dsel_{i}")
                    nc.gpsimd.affine_select(
                        out=dsel[:], in_=scr[:, off:off + 128],
                        compare_op=AL.is_equal, fill=0.0, base=0,
                        pattern=[[-1, 128]], channel_multiplier=1)
                    nc.vector.tensor_reduce(dg8[:, i:i + 1], dsel[:], AX.X, AL.add)

        # -------- tail stages, interleaved into the chunk loop --------
        dls = small.tile([64, HALF], BF, tag="dls")
        fsC = [small.tile([2, MB], FP, tag=f"fsC_{jb}", name=f"fsC_{jb}")
               for jb in range(2)]

        def tailA(jb):
            ch4 = slice(4 * jb, 4 * jb + 4)
            rs4 = small.tile([128, 4], FP, tag=f"rs4_{jb}", name=f"rs4_{jb}")
            nc.vector.tensor_reduce(
                rs4[:], rc32[:, 16 * jb:16 * jb + 16].rearrange(
                    "p (i m) -> p i m", m=4), AX.X, AL.add)
            nc.vector.reciprocal(rs4[:], rs4[:])
            nc.vector.tensor_tensor(dl[:, ch4], dg8[:, ch4], rs4[:], AL.mult)
            ptd = zps.tile([4, 128], FP, tag="zpt", name=f"ptd_{jb}",
                           padded_shape=[4, 512])
            _tp(nc, ptd[:], dl[:, ch4], ident)
            s4d = small.tile([4, 128], BF, tag=f"s4d_{jb}", name=f"s4d_{jb}")
            nc.vector.tensor_copy(s4d[:], ptd[:])
            dlrow = small.tile([1, MB], BF, tag=f"dlrow_{jb}", name=f"dlrow_{jb}")
            (nc.sync if jb == 0 else nc.scalar).dma_start(
                dlrow[:].rearrange("o (i p) -> o i p", p=128), s4d[:])
            return dlrow

        def tailB(jb, dlrow):
            hsl = HBs[jb]
            dp = zps.tile([64, MB], FP, tag="zpt", name=f"dlsb_{jb}")
            nc.tensor.matmul(dp[:], onesr64, dlrow[:], start=True, stop=True)
            nc.scalar.copy(dls[:, hsl], dp[:])
            nc.vector.tensor_tensor(hca[0:64, hsl], ph1sb[:, hsl], dls[:, hsl],
                                    AL.mult)

        def tailC(jb):
            hsl = HBs[jb]
            ph2 = zps.tile([64, MB], FP, tag="zpt", name=f"ph2_{jb}")
            nc.tensor.matmul(ph2[:], w2a, hca[:, hsl], start=True, stop=True)
            nc.vector.tensor_tensor(hcb[0:64, hsl], ph2[:], dls[:, hsl], AL.mult)

        def tailD(jb):
            hsl = HBs[jb]
            ph3 = zps.tile([64, MB], FP, tag="zpt", name=f"ph3_{jb}")
            nc.tensor.matmul(ph3[:], w3a, hcb[:, hsl], start=True, stop=True)
            nc.vector.tensor_tensor(finsq[0:64, hsl], ph3[:], dls[:, hsl], AL.mult)
            nc.vector.tensor_tensor(finsq[0:64, hsl], finsq[0:64, hsl],
                                    x1aug[0:64, hsl], AL.add)
            nc.scalar.square(finsq[64:128, hsl], finsq[0:64, hsl])
            fsp = zps.tile([2, MB], FP, tag="zpt", name=f"fsp_{jb}")
            nc.tensor.matmul(fsp[:], sel2, finsq[:, hsl], start=True, stop=True)
            (nc.vector.tensor_copy if jb == 0 else nc.scalar.copy)(
                fsC[jb][:], fsp[:])

        def tailE(jb):
            hsl = HBs[jb]
            fstT = zps.tile([128, 8], FP, tag="zpt", name=f"fstT_{jb}",
                            padded_shape=[128, 512])
            for ii in range(4):
                _tp(nc, fstT[:, 2 * ii:2 * ii + 2], fsC[jb][:, ii * 128:(ii + 1) * 128],
                    ident)
            v = fstT[:].rearrange("p (i s) -> p s i", s=2)
            mu4 = small.tile([128, 4], FP, tag=f"fmu_{jb}", name=f"fmu_{jb}")
            nc.vector.tensor_scalar(mu4[:].unsqueeze(1), v[:, 0:1, :], 1.0 / 64,
                                    None, AL.mult)
            m24 = small.tile([128, 4], FP, tag=f"fm2_{jb}", name=f"fm2_{jb}")
            nc.vector.tensor_tensor(m24[:], mu4[:], mu4[:], AL.mult)
            var4 = small.tile([128, 4], FP, tag=f"fvar_{jb}", name=f"fvar_{jb}")
            nc.vector.scalar_tensor_tensor(var4[:].unsqueeze(1), v[:, 1:2, :],
                                           1.0 / 64, m24[:].unsqueeze(1),
                                           AL.mult, AL.subtract)
            sd4 = small.tile([128, 4], FP, tag=f"fsd_{jb}", name=f"fsd_{jb}")
            nc.scalar.activation(sd4[:], var4[:], AF.Sqrt, bias=epsc128)
            fa4 = small.tile([128, 4], FP, tag=f"fa4_{jb}", name=f"fa4_{jb}")
            nc.vector.reciprocal(fa4[:], sd4[:])
            fc4 = small.tile([128, 4], FP, tag=f"fc4_{jb}", name=f"fc4_{jb}")
            nc.vector.scalar_tensor_tensor(fc4[:], mu4[:], -1.0, fa4[:],
                                           AL.mult, AL.mult)
            for ii in range(4):
                i = 4 * jb + ii
                pt = zps.tile([128, 64], BF, tag="zpt", name=f"fin_{i}",
                              padded_shape=[128, 1024])
                _tp(nc, pt[:], finsq[0:64, i * 128:(i + 1) * 128], identb)
                if ii % 2 == 0:
                    nc.vector.tensor_scalar(fin[:, i * 64:(i + 1) * 64], pt[:],
                                            fa4[:, ii:ii + 1], fc4[:, ii:ii + 1],
                                            AL.mult, AL.add)
                else:
                    nc.scalar.activation(fin[:, i * 64:(i + 1) * 64], pt[:],
                                         AF.Identity, scale=fa4[:, ii:ii + 1],
                                         bias=fc4[:, ii:ii + 1])
            f3 = fin[:, 256 * jb:256 * (jb + 1)].rearrange("p (g e) -> p g e", e=64)
            fg3 = brows["lng"][:].unsqueeze(1).broadcast_to([128, 4, 64])
            fb3 = brows["lnb"][:].unsqueeze(1).broadcast_to([128, 4, 64])
            nc.vector.tensor_tensor(f3, f3, fg3, AL.mult)
            nc.vector.tensor_tensor(f3, f3, fb3, AL.add)
            nc.sync.dma_start(
                io["outH"][512 * jb:512 * (jb + 1), :].rearrange(
                    "(i p) e -> p i e", p=128),
                fin[:, 256 * jb:256 * (jb + 1)].rearrange("p (i e) -> p i e", e=64))

        passes12(0)
        passes12(1)
        dlrow0 = None
        for i in range(HCH):
            pass3(i)
            if i + 2 < HCH:
                passes12(i + 2)
            if i == 4:
                dlrow0 = tailA(0)
            elif i == 5:
                tailB(0, dlrow0)
            elif i == 6:
                tailC(0)
            elif i == 7:
                tailD(0)
        dlrow1 = tailA(1)
        tailE(0)
        tailB(1, dlrow1)
        tailC(1)
        tailD(1)
        tailE(1)


def _build():
    if "nc" in _CACHE:
        return _CACHE["nc"]
    nc = bacc.Bacc("TRN2", target_bir_lowering=False, debug=False,
                   enable_asserts=True, num_devices=8)
    io = {}
    io["xT"] = nc.dram_tensor("xT", [G, N], BF, kind="ExternalInput").ap()
    io["lastT"] = nc.dram_tensor("lastT", [G, N], BF, kind="ExternalInput").ap()
    io["origT"] = nc.dram_tensor("origT", [E, HALF], BF, kind="ExternalInput").ap()
    io["corr4"] = nc.dram_tensor("corr4", [4, N], BF, kind="ExternalInput").ap()
    io["wpack"] = nc.dram_tensor("wpack", [128, WPACK_W], BF, kind="ExternalInput").ap()
    io["fpack"] = nc.dram_tensor("fpack", [128, FPACK_W], FP, kind="ExternalInput").ap()
    io["outH"] = nc.dram_tensor("outH", [HALF, E], FP, kind="ExternalOutput").ap()
    io["lastH"] = nc.dram_tensor("lastH", [HALF, G], FP, kind="ExternalOutput").ap()

    with tile.TileContext(nc) as tc:
        with ExitStack() as ctx:
            _emit(ctx, tc, io)
    nc.compile()
    nc.m = get_hw_module(nc.m)
    _CACHE["nc"] = nc
    return nc


def _host_prep(inputs):
    f32 = np.float32
    bf = ml_dtypes.bfloat16
    inp = {k: np.asarray(v, f32) for k, v in inputs.items()}
    ch = 1.0 + inp["mlp_w"].sum(axis=0)
    assert (ch > 0).all(), "head-mixing scale fold requires positive c_h"
    g, b = inp["bn_g"], inp["bn_b"]
    qw_c = inp["q_w"] * np.repeat(ch / np.sqrt(G), G)[None, :]
    Wq = g[:, None] * qw_c
    qA = np.concatenate([Wq, Wq.sum(axis=0)[None], (b @ qw_c)[None]], axis=0)
    Wk = g[:, None] * inp["k_w"]
    kA = np.concatenate([Wk, Wk.sum(axis=0)[None], (b @ inp["k_w"])[None]], axis=0)
    w1 = inp["gcn_w1"]
    w1a = np.concatenate([w1, -(inp["gcn_b3"] @ w1)[None]], axis=0)
    w2a = np.concatenate([inp["gcn_w2"], (inp["gcn_b1"] @ inp["gcn_w2"])[None]], axis=0)
    w3a = np.concatenate([inp["gcn_w3"], (inp["gcn_b2"] @ inp["gcn_w3"])[None]], axis=0)
    fc3a = np.concatenate([inp["fc3_w"], inp["fc3_b"][None, :]], axis=0)

    wpack = np.zeros((128, WPACK_W), f32)
    wpack[0:128, W_IDB:W_IDB + 128] = np.eye(128)
    wpack[0:128, W_WZ:W_WZ + 64] = inp["w_z"]
    wpack[0:128, W_WR:W_WR + 64] = inp["w_r"]
    wpack[0:128, W_WH:W_WH + 64] = inp["w_h"]
    wpack[0:66, W_QA:W_QA + 256] = qA
    wpack[0:66, W_KA:W_KA + 256] = kA
    wpack[0:64, W_FC1:W_FC1 + 16] = inp["fc1_w"]
    wpack[0:16, W_FC2:W_FC2 + 2] = inp["fc2_w"]
    wpack[0:3, W_FC3A:W_FC3A + 64] = fc3a
    wpack[0:65, W_W1A:W_W1A + 64] = w1a
    wpack[0:65, W_W2A:W_W2A + 64] = w2a
    wpack[0:65, W_W3A:W_W3A + 64] = w3a
    wpack[0:64, W_SEL:W_SEL + 1] = 1.0
    wpack[64:128, W_SEL + 1:W_SEL + 2] = 1.0
    wpack[:, W_ONE:W_ONE + 128] = 1.0

    fpack = np.zeros((128, FPACK_W), f32)
    fpack[0:128, F_IDF:F_IDF + 128] = np.eye(128)
    fpack[0:16, F_B + 0] = inp["fc1_b"]
    fpack[0:2, F_B + 1] = inp["fc2_b"]
    fpack[0:64, F_B + 2] = inp["fc3_b"]
    fpack[0:128, F_EPS] = EPS
    fpack[0:64, F_XG] = inp["x_nom_g"]
    fpack[0:64, F_XB3] = inp["x_nom_b"] + inp["gcn_b3"]
    for k, nm in enumerate(("bn_g", "bn_b", "last_nom_g", "last_nom_b")):
        fpack[0, F_BN + 64 * k:F_BN + 64 * (k + 1)] = inp[nm]

    def c(a, dt=bf):
        return np.ascontiguousarray(np.asarray(a, dt))

    shared = {"wpack": c(wpack), "fpack": c(fpack, f32)}
    in_maps = []
    for core in range(8):
        bi, h = core // 2, core % 2
        off = h * HALF
        corr4 = np.stack([
            np.roll(inp["attn_norm_g"], -off),
            np.roll(inp["skip_norm_g"], -off),
            np.roll(inp["attn_norm_b"] + inp["skip_norm_b"], -off),
            np.ones(N, f32),
        ])
        m = dict(shared)
        m["xT"] = c(np.roll(inp["x"][bi], -off, axis=0).T)
        m["lastT"] = c(np.roll(inp["last_G_emb"][bi], -off, axis=0).T)
        m["origT"] = c(inp["orig_x"][bi, off:off + HALF].T)
        m["corr4"] = c(corr4)
        in_maps.append(m)
    return in_maps


def run(inputs, trace=False):
    nc = _build()
    in_maps = _host_prep(inputs)
    res = run_bass_kernel_spmd(nc, in_maps, core_ids=list(range(8)), trace=trace)
    out = np.zeros((B, N, E), np.float32)
    last = np.zeros((B, N, G), np.float32)
    for core in range(8):
        bi, h = core // 2, core % 2
        off = h * HALF
        out[bi, off:off + HALF] = res.results[core]["outH"]
        last[bi, off:off + HALF] = res.results[core]["lastH"]
    return (out, last), res


def kernel(**inputs):
    return run(inputs)[0]
